# revision 13
# baseline (speedup 1.0000x reference)
"""Trainium2 Bass kernel for nn_AttentionBlock (dense transformer block).

Strategy (8 NeuronCores, one chip):
  - Attention head-parallel: core c owns heads {2c, 2c+1} for all batches;
    computes Q/K/V projections (only its 2 heads), scores^T, softmax (exp on
    ScalarE, denominator via ones-column in the V matmul, DVE reciprocal),
    and the normalized context ctx^T.
  - AllToAll redistributes ctx^T shards so core c gets ALL heads for its
    S/8 = 256-query token shard.
  - Token-parallel back half: WO + residual + LN1 + MLP(relu) + residual +
    LN2 for the core's 1024 tokens (4 batches x 256 queries).
  All data flows feature-major ("transposed"): tiles are [feature_partition,
  token_free], so every matmul contraction runs on the partition dim.

Perf structure (v1):
  - Scores for head0 (partitions 0:64) and head1 (64:128) are emitted as
    adjacent matmuls -> PE row-tiling (tile_position (0,0)/(64,0)) runs
    them concurrently in the array: scores PE time halves.
  - Attention is sc-major; both heads' ctx chains trail the scores by one
    l-chunk; softmax normalization (reciprocal broadcast + stage multiply)
    is pipelined one sc-group behind so the PE never waits on the DVE
    reciprocal chain.
  - proj is double-buffered; QKV projection of batch b+1 interleaves into
    the exp-bound attention of batch b; V-transposes of b+1 run in the
    inter-batch gap; WO of back-half chunk0 interleaves into batch 3.
  - LayerNorm sum/sumsq chains are col-tiled into one PSUM tile (rows 0
    and 32 -> concurrent in the array); Square/ReLU run on DVE so ScalarE
    does exp (the attention pacer) only.
"""
import numpy as np

import concourse.bass as bass
import concourse.tile as tile
from concourse import mybir, bacc
from concourse.masks import make_identity

F32 = mybir.dt.float32
F32R = mybir.dt.float32r
BF16 = mybir.dt.bfloat16
AF = mybir.ActivationFunctionType
ALU = mybir.AluOpType

S, B, DM, H, DFF = 2048, 4, 1024, 16, 4096
DK = DM // H  # 64
NC = 8
SS = S // NC  # 256: seq shard per core
T = B * SS  # 1024 tokens per core in the back half
EPS = 1e-5

_CACHE = {}


def _build_nc():
    nc = bacc.Bacc("TRN2", target_bir_lowering=False, debug=False, num_devices=NC)

    # ---------------- I/O ----------------
    xt = nc.declare_dram_parameter("xt", [DM, B, S], BF16, isOutput=False)
    xts = nc.declare_dram_parameter("xts", [DM, B, SS], F32, isOutput=False)
    wqt = nc.declare_dram_parameter("wqt", [8, 128, 128], BF16, isOutput=False)
    wkt = nc.declare_dram_parameter("wkt", [8, 128, 128], BF16, isOutput=False)
    wvt = nc.declare_dram_parameter("wvt", [8, 128, 128], BF16, isOutput=False)
    bqkv = nc.declare_dram_parameter("bqkv", [128, 3], F32, isOutput=False)
    wotr = nc.declare_dram_parameter("wotr", [8, 8, 128, 128], BF16, isOutput=False)
    wob = nc.declare_dram_parameter("wob", [128, 8], F32, isOutput=False)
    w1tr = nc.declare_dram_parameter("w1tr", [32, 8, 128, 128], F32R, isOutput=False)
    b1 = nc.declare_dram_parameter("b1", [128, 32], F32, isOutput=False)
    w2tr = nc.declare_dram_parameter("w2tr", [8, 32, 128, 128], BF16, isOutput=False)
    b2 = nc.declare_dram_parameter("b2", [128, 8], F32, isOutput=False)
    ln1g = nc.declare_dram_parameter("ln1g", [128, 8], F32, isOutput=False)
    ln1b = nc.declare_dram_parameter("ln1b", [128, 8], F32, isOutput=False)
    ln2g = nc.declare_dram_parameter("ln2g", [128, 8], F32, isOutput=False)
    ln2b = nc.declare_dram_parameter("ln2b", [128, 8], F32, isOutput=False)
    ot = nc.declare_dram_parameter("ot", [DM, B, SS], F32, isOutput=True)

    with tile.TileContext(nc) as tc, nc.allow_low_precision(
        reason="float32r matmul operands (TF32-like) are intentional"
    ):
        with (
            tc.tile_pool(name="dram", bufs=1, space="DRAM") as dram,
            tc.tile_pool(name="const", bufs=1) as const,
            tc.tile_pool(name="wres", bufs=1) as wres,
            tc.tile_pool(name="qkv", bufs=1) as qkvp,
            tc.tile_pool(name="gen", bufs=2) as gen,
            tc.tile_pool(name="ps", bufs=2, space="PSUM") as ps,
        ):
            a2a_in = []
            a2a_out = []
            for b in range(B):
                ai = dram.tile([NC, 2, DK, SS], BF16, tag=f"a2a_in{b}",
                               name=f"a2a_in{b}")
                ao = dram.tile([NC, 2, DK, SS], BF16, tag=f"a2a_out{b}",
                               name=f"a2a_out{b}")
                a2a_in.append(ai)
                a2a_out.append(ao)

            # ---------------- constants ----------------
            ident = const.tile([128, 128], BF16, tag="ident")
            make_identity(nc, ident[:])
            ones_1x64 = const.tile([1, 64], BF16, tag="ones_1x64")
            nc.gpsimd.memset(ones_1x64[:], 1.0)
            ones_1x128 = const.tile([1, 128], F32R, tag="ones_1x128")
            scr_r = gen.tile([1, 128], F32, tag="stat", bufs=3,
                             name="ones_scr_r")
            nc.vector.memset(scr_r[:], 1.0)
            nc.vector.tensor_copy(out=ones_1x128[:], in_=scr_r[:])
            ones_128x1 = const.tile([128, 1], F32R, tag="ones_128x1")
            scr_c = gen.tile([128, 1], F32, tag="lnw", bufs=3,
                             name="ones_scr_c")
            nc.vector.memset(scr_c[:], 1.0)
            nc.vector.tensor_copy(out=ones_128x1[:], in_=scr_c[:])
            eps_sb = const.tile([1, 1], F32, tag="eps")
            nc.vector.memset(eps_sb[:], EPS)
            bqkv_sb = const.tile([128, 3], F32, tag="bqkv")
            nc.sync.dma_start(out=bqkv_sb[:], in_=bqkv[:, :])
            wob_sb = const.tile([128, 8], F32, tag="wob")
            nc.sync.dma_start(out=wob_sb[:], in_=wob[:, :])
            b1_sb = const.tile([128, 32], F32, tag="b1")
            nc.sync.dma_start(out=b1_sb[:], in_=b1[:, :])
            b2_sb = const.tile([128, 8], F32, tag="b2")
            nc.sync.dma_start(out=b2_sb[:], in_=b2[:, :])
            ln_sb = {}
            for name, h in (("ln1g", ln1g), ("ln1b", ln1b), ("ln2g", ln2g),
                            ("ln2b", ln2b)):
                t_ = const.tile([128, 8], F32, tag=name)
                nc.sync.dma_start(out=t_[:], in_=h[:, :])
                ln_sb[name] = t_

            # resident QKV weights: [128p, 8ic, 128(2h dk)]
            w_sb = {}
            for name, h in (("wq", wqt), ("wk", wkt), ("wv", wvt)):
                t_ = wres.tile([128, 8, 128], BF16, tag=name)
                nc.sync.dma_start(
                    out=t_[:], in_=h[:, :, :].rearrange("ic p j -> p ic j")
                )
                w_sb[name] = t_
            # warm start: first attention rhs tiles, issued before anything
            warm_xt = []
            for ic in range(8):
                x_ = gen.tile([128, 512], BF16, tag="xtin", bufs=8,
                              name=f"warm_xt{ic}")
                nc.sync.dma_start(
                    out=x_[:], in_=xt[ic * 128:(ic + 1) * 128, 0, 0:512])
                warm_xt.append(x_)

            # early prefetch of back-half weight streams (no data deps)
            pre_wo = {}
            pre_w1 = {}
            pre_w2 = {}
            for oc in range(2):
                t_ = gen.tile([128, 8, 128], BF16, tag="wo_c", bufs=2,
                              name=f"pre_wo{oc}")
                nc.gpsimd.dma_start(
                    out=t_[:],
                    in_=wotr[oc, :, :, :].rearrange("ic p j -> p ic j"))
                pre_wo[oc] = t_
            for fc in range(3):
                t_ = gen.tile([128, 8, 128], F32R, tag="w1_c", bufs=3,
                              name=f"pre_w1{fc}")
                nc.gpsimd.dma_start(
                    out=t_[:],
                    in_=w1tr[fc, :, :, :].rearrange("ic p j -> p ic j"))
                pre_w1[fc] = t_
            for half in range(2):
                t_ = gen.tile([128, 16, 128], BF16, tag="w2_c", bufs=2,
                              name=f"pre_w2{half}")
                nc.gpsimd.dma_start(
                    out=t_[:],
                    in_=w2tr[0, half * 16:(half + 1) * 16, :, :]
                    .rearrange("fc p j -> p fc j"))
                pre_w2[half] = t_

            # persistent double-buffered per-batch state
            proj_sets = []
            for st in range(2):
                d = {}
                for name in ("wq", "wk", "wv"):
                    d[name] = qkvp.tile([128, S], BF16, tag=f"{name}{st}",
                                        name=f"{name}_s{st}")
                proj_sets.append(d)
            vt_sets = []
            for st in range(2):
                vts = []
                for lc in range(16):
                    v_ = const.tile([128, 130], BF16, tag=f"vt{st}_{lc}")
                    nc.gpsimd.memset(v_[:, 64:65], 1.0)
                    nc.gpsimd.memset(v_[:, 129:130], 1.0)
                    vts.append(v_)
                vt_sets.append(vts)

            # ---------- emission helpers ----------
            def load_xt(b, sc):
                if b == 0 and sc == 0:
                    return warm_xt
                xt_t = []
                for ic in range(8):
                    x_ = gen.tile([128, 512], BF16, tag="xtin", bufs=8)
                    nc.sync.dma_start(
                        out=x_[:],
                        in_=xt[ic * 128:(ic + 1) * 128, b,
                               sc * 512:(sc + 1) * 512],
                    )
                    xt_t.append(x_)
                return xt_t

            def emit_p1_proj(b, sc, pi, name, xt_t):
                """One projection chain (wq/wk/wv) for batch b chunk sc."""
                proj = proj_sets[b % 2]
                pmm = ps.tile([128, 512], F32, tag="pmm", bufs=2,
                              name=f"p1_{b}_{sc}_{name}")
                for ic in range(8):
                    nc.tensor.matmul(
                        out=pmm[:],
                        lhsT=w_sb[name][:, ic, :],
                        rhs=xt_t[ic][:],
                        start=(ic == 0),
                        stop=(ic == 7),
                    )
                nc.vector.tensor_scalar_add(
                    out=proj[name][:, sc * 512:(sc + 1) * 512],
                    in0=pmm[:],
                    scalar1=bqkv_sb[:, pi:pi + 1],
                )

            def emit_p2_chunk(b, lc):
                """Transpose one 128-token V chunk into vt tiles."""
                proj = proj_sets[b % 2]
                vts = vt_sets[b % 2]
                ptr = ps.tile([128, 128], BF16, tag="pb1", bufs=2,
                              padded_shape=[128, 512])
                nc.tensor.transpose(
                    out=ptr[:],
                    in_=proj["wv"][:, lc * 128:(lc + 1) * 128],
                    identity=ident[:],
                )
                for hl in range(2):
                    nc.vector.tensor_copy(
                        out=vts[lc][:, hl * 65:hl * 65 + 64],
                        in_=ptr[:, hl * 64:(hl + 1) * 64],
                    )

            def emit_softmax_norm(b, sc, pctx, recips):
                """Reciprocal broadcast + stage multiply + a2a shard DMAs
                for group (b, sc).  Called one group late so the PE's pbc
                matmul never waits on the DVE reciprocal chain."""
                for hl in range(2):
                    nc.tensor.matmul(
                        out=pctx[hl][64:128, :],
                        lhsT=ones_1x64[:],
                        rhs=recips[hl][:],
                        start=True,
                        stop=True,
                    )
                for hl in range(2):
                    bc = gen.tile([64, 512], BF16, tag="bc", bufs=1)
                    nc.vector.tensor_copy(out=bc[:], in_=pctx[hl][64:128, :])
                    stage = gen.tile([64, 512], BF16, tag="stage", bufs=2)
                    nc.vector.tensor_tensor(
                        out=stage[:], in0=pctx[hl][0:64, :], in1=bc[:],
                        op=ALU.mult,
                    )
                    for half in range(2):
                        d = sc * 2 + half
                        nc.sync.dma_start(
                            out=a2a_in[b][d, hl, :, :],
                            in_=stage[:, half * 256:(half + 1) * 256],
                        )

            def emit_attn_sc(b, sc, extra, pending):
                """Both heads' scores/exp/ctx for group (b, sc); returns
                the group's (pctx, recips) for deferred normalization.

                extra: dict lc2 -> list of emission thunks (PE filler).
                pending: previous group's deferred normalization (emitted
                at lc2==0, after this group's first score pair)."""
                proj = proj_sets[b % 2]
                vts = vt_sets[b % 2]
                exp_t = [[], []]
                pctx = [None, None]
                for lc2 in range(8):
                    psc = [None, None]
                    for hl in range(2):
                        psc[hl] = ps.tile([128, 1024], F32, tag="psc",
                                          bufs=2,
                                          name=f"psc{b}_{sc}_{lc2}_{hl}")
                    for k in range(2):
                        for hl in range(2):
                            hb = hl * 64
                            nc.tensor.matmul(
                                out=psc[hl][:, k * 512:(k + 1) * 512],
                                lhsT=proj["wk"][hb:hb + 64,
                                                (lc2 * 2 + k) * 128:
                                                (lc2 * 2 + k + 1) * 128],
                                rhs=proj["wq"][hb:hb + 64,
                                               sc * 512:(sc + 1) * 512],
                                start=True,
                                stop=True,
                            )
                    if lc2 == 0 and pending is not None:
                        emit_softmax_norm(*pending)
                    for hl in range(2):
                        e_ = gen.tile([128, 1024], BF16, tag="u2k", bufs=16,
                                      name=f"e{b}_{sc}_{lc2}_{hl}")
                        nc.scalar.activation(
                            out=e_[:], in_=psc[hl][:], func=AF.Exp
                        )
                        exp_t[hl].append(e_)
                    if lc2 == 1:
                        for hl in range(2):
                            pctx[hl] = ps.tile(
                                [128, 512], F32, tag="pb1", bufs=2,
                                name=f"pctx{b}_{sc}_{hl}")
                    if lc2 >= 1:
                        src = lc2 - 1
                        for hl in range(2):
                            for k in range(2):
                                lc = src * 2 + k
                                nc.tensor.matmul(
                                    out=pctx[hl][0:65, :],
                                    lhsT=vts[lc][:, hl * 65:hl * 65 + 65],
                                    rhs=exp_t[hl][src][:, k * 512:
                                                       (k + 1) * 512],
                                    start=(lc == 0),
                                    stop=False,
                                )
                    for th in extra.get(lc2, ()):
                        th()
                # finish both ctx chains (src=7)
                for hl in range(2):
                    for k in range(2):
                        lc = 14 + k
                        nc.tensor.matmul(
                            out=pctx[hl][0:65, :],
                            lhsT=vts[lc][:, hl * 65:hl * 65 + 65],
                            rhs=exp_t[hl][7][:, k * 512:(k + 1) * 512],
                            start=False,
                            stop=(lc == 15),
                        )
                # denominators -> reciprocals (DVE; consumed next group)
                recips = [None, None]
                for hl in range(2):
                    dsum = gen.tile([1, 512], F32, tag="dsum", bufs=1)
                    nc.vector.tensor_copy(out=dsum[:],
                                          in_=pctx[hl][64:65, :])
                    rf = gen.tile([1, 512], F32, tag="rf32", bufs=1)
                    nc.vector.reciprocal_approx_fast(out=rf[:], in_=dsum[:])
                    recips[hl] = gen.tile([1, 512], BF16, tag="recip",
                                          bufs=2, name=f"rcp{b}_{sc}_{hl}")
                    nc.vector.tensor_copy(out=recips[hl][:], in_=rf[:])
                return pctx, recips

            # =====================================================
            # Back-half emission helpers
            # =====================================================
            def ln_stats(rt_tiles, tagn):
                # sum (row 0) and sumsq (row 32) chains col-tiled into one
                # PSUM tile -> concurrent in the PE array
                psum_s = ps.tile([1, 512], F32, tag="pb1", bufs=2,
                                 padded_shape=[128, 512],
                                 name=f"psum_s_{tagn}")
                sq_t = []
                for ic in range(8):
                    sq = gen.tile([128, 512], F32R, tag="sq", bufs=2,
                                  name=f"sq_{tagn}_{ic}")
                    nc.vector.tensor_tensor(out=sq[:], in0=rt_tiles[ic][:],
                                            in1=rt_tiles[ic][:],
                                            op=ALU.mult)
                    sq_t.append(sq)
                for ic in range(8):
                    nc.tensor.matmul(
                        out=psum_s[:], lhsT=ones_128x1[:],
                        rhs=rt_tiles[ic][:],
                        start=(ic == 0), stop=(ic == 7),
                    )
                psum_q = ps.tile([1, 512], F32, tag="pb1", bufs=2,
                                 padded_shape=[128, 512],
                                 name=f"psum_q_{tagn}")
                for ic in range(8):
                    nc.tensor.matmul(
                        out=psum_q[:], lhsT=ones_128x1[:],
                        rhs=sq_t[ic][:],
                        start=(ic == 0), stop=(ic == 7),
                    )
                mu = gen.tile([1, 512], F32R, tag="mu", bufs=2,
                              name=f"mu_{tagn}")
                nc.vector.tensor_scalar_mul(out=mu[:], in0=psum_s[:],
                                            scalar1=1.0 / DM)
                ex2 = gen.tile([1, 512], F32, tag="stat", bufs=3,
                               name=f"ex2_{tagn}")
                nc.vector.tensor_scalar_mul(out=ex2[:], in0=psum_q[:],
                                            scalar1=1.0 / DM)
                musq = gen.tile([1, 512], F32, tag="stat", bufs=3,
                                name=f"musq_{tagn}")
                nc.vector.tensor_tensor(out=musq[:], in0=mu[:], in1=mu[:],
                                        op=ALU.mult)
                var = gen.tile([1, 512], F32, tag="stat", bufs=3,
                               name=f"var_{tagn}")
                nc.vector.tensor_tensor(out=var[:], in0=ex2[:], in1=musq[:],
                                        op=ALU.subtract)
                sd = gen.tile([1, 512], F32, tag="stat", bufs=3,
                              name=f"sd_{tagn}")
                nc.scalar.activation(out=sd[:], in_=var[:], func=AF.Sqrt,
                                     bias=eps_sb[:])
                rscr = gen.tile([1, 512], F32, tag="stat", bufs=3,
                                name=f"rscr_{tagn}")
                rf = gen.tile([1, 512], F32, tag="stat", bufs=3,
                              name=f"rf_{tagn}")
                nc.vector.reciprocal_approx_accurate(out=rf[:], in_=sd[:],
                                                     scratch=rscr[:])
                rstd = gen.tile([1, 512], F32R, tag="rstd", bufs=2,
                                name=f"rstd_{tagn}")
                nc.vector.tensor_copy(out=rstd[:], in_=rf[:])
                return mu, rstd

            def ln_norm(stats, rt_tiles, g_sb, b_sb, out_dtype, fold_gb,
                        tagn, out_cb=None):
                mu, rstd = stats
                pmu = ps.tile([128, 512], F32, tag="pb1", bufs=2,
                              name=f"pmu_{tagn}")
                nc.tensor.matmul(out=pmu[:], lhsT=ones_1x128[:], rhs=mu[:],
                                 start=True, stop=True)
                prs = ps.tile([128, 512], F32, tag="pb1", bufs=2,
                              name=f"prs_{tagn}")
                nc.tensor.matmul(out=prs[:], lhsT=ones_1x128[:], rhs=rstd[:],
                                 start=True, stop=True)
                outs = []
                for ic in range(8):
                    tmp = gen.tile([128, 512], F32, tag="lnw", bufs=3,
                                   name=f"tmp_{tagn}_{ic}")
                    nc.vector.tensor_tensor(out=tmp[:], in0=rt_tiles[ic][:],
                                            in1=pmu[:], op=ALU.subtract)
                    if fold_gb:
                        o_ = gen.tile([128, 512], out_dtype, tag="u2k",
                                      bufs=16, name=f"z_{tagn}_{ic}")
                        nc.vector.tensor_tensor(out=o_[:], in0=tmp[:],
                                                in1=prs[:], op=ALU.mult)
                    else:
                        a_ = gen.tile([128, 512], F32, tag="lnw", bufs=3,
                                      name=f"a_{tagn}_{ic}")
                        nc.vector.tensor_scalar_mul(
                            out=a_[:], in0=prs[:],
                            scalar1=g_sb[:, ic:ic + 1])
                        t2 = gen.tile([128, 512], F32, tag="lnw", bufs=3,
                                      name=f"t2_{tagn}_{ic}")
                        nc.vector.tensor_tensor(out=t2[:], in0=tmp[:],
                                                in1=a_[:], op=ALU.mult)
                        o_ = gen.tile([128, 512], out_dtype, tag="res8",
                                      bufs=4, name=f"o_{tagn}_{ic}")
                        nc.vector.tensor_scalar_add(
                            out=o_[:], in0=t2[:],
                            scalar1=b_sb[:, ic:ic + 1])
                        if out_cb is not None:
                            out_cb(ic, o_)
                    outs.append(o_)
                return outs

            def emit_wo_oc(t2c, oc, ctx_t, r1_t):
                """One 128-feature output chunk of WO + residual."""
                if t2c == 0 and oc in pre_wo:
                    wo_c = pre_wo.pop(oc)
                else:
                    wo_c = gen.tile([128, 8, 128], BF16, tag="wo_c",
                                    bufs=2, name=f"wo_c_{t2c}_{oc}")
                    nc.sync.dma_start(
                        out=wo_c[:],
                        in_=wotr[oc, :, :, :].rearrange(
                            "ic p j -> p ic j"),
                    )
                pmm = ps.tile([128, 512], F32, tag="pmm", bufs=2,
                              name=f"pwo_{t2c}_{oc}")
                for ic in range(8):
                    nc.tensor.matmul(
                        out=pmm[:], lhsT=wo_c[:, ic, :],
                        rhs=ctx_t[ic][:],
                        start=(ic == 0), stop=(ic == 7),
                    )
                x_ = gen.tile([128, 512], F32, tag="xres", bufs=2,
                              name=f"x_{t2c}_{oc}")
                b0 = t2c * 2
                nc.sync.dma_start(
                    out=x_[:],
                    in_=xts[oc * 128:(oc + 1) * 128,
                            b0:b0 + 2, :].rearrange("p b s -> p (b s)"),
                )
                r1 = gen.tile([128, 512], F32R, tag="r1t", bufs=9,
                              name=f"r1_{t2c}_{oc}")
                nc.vector.scalar_tensor_tensor(
                    out=r1[:], in0=pmm[:], scalar=wob_sb[:, oc:oc + 1],
                    in1=x_[:], op0=ALU.add, op1=ALU.add,
                )
                r1_t.append(r1)

            def load_ctx(t2c):
                b0 = t2c * 2
                ctx_t = []
                for ic in range(8):
                    c_ = gen.tile([128, 512], BF16, tag="ctx", bufs=8,
                                  name=f"c_{t2c}_{ic}")
                    for half in range(2):
                        nc.sync.dma_start(
                            out=c_[:, half * 256:(half + 1) * 256],
                            in_=a2a_out[b0 + half][ic, :, :, :].rearrange(
                                "hl v s -> (hl v) s"
                            ),
                        )
                    ctx_t.append(c_)
                return ctx_t

            def emit_mlp1(t2c, x1_t):
                h_t = []
                for fc in range(32):
                    if t2c == 0 and fc in pre_w1:
                        w1_c = pre_w1.pop(fc)
                    else:
                        w1_c = gen.tile([128, 8, 128], F32R, tag="w1_c",
                                        bufs=3, name=f"w1_c_{t2c}_{fc}")
                        nc.sync.dma_start(
                            out=w1_c[:],
                            in_=w1tr[fc, :, :, :].rearrange(
                                "ic p j -> p ic j"),
                        )
                    pmm = ps.tile([128, 512], F32, tag="pmm", bufs=2,
                                  name=f"ph_{t2c}_{fc}")
                    for ic in range(8):
                        nc.tensor.matmul(
                            out=pmm[:], lhsT=w1_c[:, ic, :], rhs=x1_t[ic][:],
                            start=(ic == 0), stop=(ic == 7),
                        )
                    h_ = gen.tile([128, 512], BF16, tag="ht", bufs=32,
                                  name=f"h_{t2c}_{fc}")
                    nc.vector.tensor_scalar(
                        out=h_[:], in0=pmm[:],
                        scalar1=b1_sb[:, fc:fc + 1], scalar2=0.0,
                        op0=ALU.add, op1=ALU.max,
                    )
                    h_t.append(h_)
                return h_t

            def emit_mlp2(t2c, h_t, x1_t):
                r2_t = []
                for oc in range(8):
                    pmm = ps.tile([128, 512], F32, tag="pmm", bufs=2,
                                  name=f"pm_{t2c}_{oc}")
                    for half in range(2):
                        if t2c == 0 and oc == 0 and half in pre_w2:
                            w2_c = pre_w2.pop(half)
                        else:
                            w2_c = gen.tile(
                                [128, 16, 128], BF16, tag="w2_c", bufs=2,
                                name=f"w2_c_{t2c}_{oc}_{half}")
                            nc.sync.dma_start(
                                out=w2_c[:],
                                in_=w2tr[oc, half * 16:(half + 1) * 16, :, :]
                                .rearrange("fc p j -> p fc j"),
                            )
                        for f2 in range(16):
                            fc = half * 16 + f2
                            nc.tensor.matmul(
                                out=pmm[:], lhsT=w2_c[:, f2, :],
                                rhs=h_t[fc][:],
                                start=(fc == 0), stop=(fc == 31),
                            )
                    r2p = gen.tile([128, 512], F32, tag="lnw", bufs=3,
                                   name=f"r2p_{t2c}_{oc}")
                    nc.vector.scalar_tensor_tensor(
                        out=r2p[:], in0=x1_t[oc][:],
                        scalar=ln_sb["ln1g"][:, oc:oc + 1],
                        in1=pmm[:], op0=ALU.mult, op1=ALU.add,
                    )
                    r2 = gen.tile([128, 512], F32R, tag="u2k", bufs=16,
                                  name=f"r2_{t2c}_{oc}")
                    nc.vector.tensor_scalar_add(
                        out=r2[:], in0=r2p[:], scalar1=b2_sb[:, oc:oc + 1],
                    )
                    r2_t.append(r2)
                return r2_t

            def emit_out(t2c, o_t):
                b0 = t2c * 2
                for oc in range(8):
                    nc.sync.dma_start(
                        out=ot[oc * 128:(oc + 1) * 128,
                               b0:b0 + 2, :].rearrange("p b s -> p (b s)"),
                        in_=o_t[oc][:],
                    )

            # =====================================================
            # Main schedule
            # =====================================================
            # Prologue: full QKV + V-transpose for batch 0
            for sc in range(4):
                xt_t = load_xt(0, sc)
                for pi, name in enumerate(("wq", "wk", "wv")):
                    emit_p1_proj(0, sc, pi, name, xt_t)
                for lc in range(4 * sc, 4 * sc + 4):
                    emit_p2_chunk(0, lc)

            ctx0_t = None
            r1_0 = []
            pending = None
            for b in range(B):
                for sc in range(4):
                    extra = {}
                    if b < 3:
                        # QKV of batch b+1, spread across this group
                        xt_box = []

                        def load_thunk(b=b, sc=sc, box=xt_box):
                            box.append(load_xt(b + 1, sc))

                        extra[0] = [load_thunk]
                        extra[1] = [lambda b=b, sc=sc, box=xt_box:
                                    emit_p1_proj(b + 1, sc, 0, "wq", box[0])]
                        extra[3] = [lambda b=b, sc=sc, box=xt_box:
                                    emit_p1_proj(b + 1, sc, 1, "wk", box[0])]
                        extra[5] = [lambda b=b, sc=sc, box=xt_box:
                                    emit_p1_proj(b + 1, sc, 2, "wv", box[0])]
                    else:
                        # WO of back-half chunk0 (a2a 0/1 done long ago)
                        extra[1] = [lambda sc=sc:
                                    emit_wo_oc(0, 2 * sc, ctx0_t, r1_0)]
                        extra[5] = [lambda sc=sc:
                                    emit_wo_oc(0, 2 * sc + 1, ctx0_t, r1_0)]
                    pctx, recips = emit_attn_sc(b, sc, extra, pending)
                    pending = (b, sc, pctx, recips)
                # flush last group's softmax before the a2a
                emit_softmax_norm(*pending)
                pending = None
                nc.gpsimd.collective_compute(
                    "AllToAll",
                    ALU.bypass,
                    replica_groups=[list(range(NC))],
                    ins=[a2a_in[b][:].opt()],
                    outs=[a2a_out[b][:].opt()],
                )
                if b < 3:
                    # V-transposes for batch b+1 in the inter-batch gap
                    for lc in range(16):
                        emit_p2_chunk(b + 1, lc)
                if b == 1:
                    # ctx loads for back-half chunk0 (consumed in batch 3)
                    ctx0_t = load_ctx(0)

            # Back half: chunk0 LN1+MLP covers a2a(b3) latency, then chunk1
            st1_0 = ln_stats(r1_0, "l1c0")
            x1_0 = ln_norm(st1_0, r1_0, None, None, F32R, True, "l1c0")
            h_0 = emit_mlp1(0, x1_0)
            ctx1_t = load_ctx(1)
            r1_1 = []
            for oc in range(8):
                emit_wo_oc(1, oc, ctx1_t, r1_1)
            st1_1 = ln_stats(r1_1, "l1c1")
            x1_1 = ln_norm(st1_1, r1_1, None, None, F32R, True, "l1c1")
            r2_0 = emit_mlp2(0, h_0, x1_0)
            st2_0 = ln_stats(r2_0, "l2c0")
            h_1 = emit_mlp1(1, x1_1)
            def out_cb0(ic, o_):
                nc.sync.dma_start(
                    out=ot[ic * 128:(ic + 1) * 128,
                           0:2, :].rearrange("p b s -> p (b s)"),
                    in_=o_[:],
                )
            ln_norm(st2_0, r2_0, ln_sb["ln2g"], ln_sb["ln2b"], F32,
                    False, "l2c0", out_cb=out_cb0)
            r2_1 = emit_mlp2(1, h_1, x1_1)
            st2_1 = ln_stats(r2_1, "l2c1")
            def out_cb1(ic, o_):
                nc.sync.dma_start(
                    out=ot[ic * 128:(ic + 1) * 128,
                           2:4, :].rearrange("p b s -> p (b s)"),
                    in_=o_[:],
                )
            ln_norm(st2_1, r2_1, ln_sb["ln2g"], ln_sb["ln2b"], F32,
                    False, "l2c1", out_cb=out_cb1)

    nc.compile()
    return nc


# ------------------------------------------------------------------
# Host side
# ------------------------------------------------------------------
def _get_runner():
    if "runner" in _CACHE:
        return _CACHE["runner"]
    import jax
    from jax.sharding import Mesh, PartitionSpec
    try:
        from jax.experimental.shard_map import shard_map
    except ImportError:
        from jax.shard_map import shard_map
    from concourse import bass2jax
    from concourse.bass2jax import _bass_exec_p, install_neuronx_cc_hook

    nc = _build_nc()
    install_neuronx_cc_hook()
    partition_name = nc.partition_id_tensor.name if nc.partition_id_tensor else None
    in_names, out_names, out_avals, zero_outs = [], [], [], []
    for alloc in nc.m.functions[0].allocations:
        if not isinstance(alloc, mybir.MemoryLocationSet):
            continue
        name = alloc.memorylocations[0].name
        if alloc.kind == "ExternalInput":
            if name != partition_name:
                in_names.append(name)
        elif alloc.kind == "ExternalOutput":
            out_names.append(name)
            shape = tuple(alloc.tensor_shape)
            dtype = mybir.dt.np(alloc.dtype)
            out_avals.append(jax.core.ShapedArray(shape, dtype))
            zero_outs.append(np.zeros(shape, dtype))
    n_params = len(in_names)
    all_in_names = list(in_names) + list(out_names)
    if partition_name is not None:
        all_in_names.append(partition_name)

    def _body(*args):
        operands = list(args)
        if partition_name is not None:
            operands.append(bass2jax.partition_id_tensor())
        outs = _bass_exec_p.bind(
            *operands,
            out_avals=tuple(out_avals),
            in_names=tuple(all_in_names),
            out_names=tuple(out_names),
            lowering_input_output_aliases=(),
            sim_require_finite=True,
            sim_require_nnan=True,
            nc=nc,
        )
        return tuple(outs)

    donate = tuple(range(n_params, n_params + len(out_names)))
    devices = jax.devices()[:NC]
    mesh = Mesh(np.asarray(devices), ("core",))
    in_specs = (PartitionSpec("core"),) * (n_params + len(out_names))
    out_specs = (PartitionSpec("core"),) * len(out_names)
    fn = jax.jit(
        shard_map(_body, mesh=mesh, in_specs=in_specs, out_specs=out_specs,
                  check_rep=False),
        donate_argnums=donate, keep_unused=True,
    )

    class R:
        pass

    r = R()
    r.fn = fn
    r.in_names = in_names
    r.out_names = out_names
    r.out_avals = out_avals
    _CACHE["runner"] = r
    return r


def _prep_in_maps(X, WQ_w, WQ_b, WK_w, WK_b, WV_w, WV_b, WO_w, WO_b,
                  ln1_g, ln1_b, W1, b1, W2, b2, ln2_g, ln2_b):
    import ml_dtypes
    f = np.float32
    bf = ml_dtypes.bfloat16
    XT = np.ascontiguousarray(X.transpose(2, 1, 0)).astype(f)  # [DM,B,S]
    wotr = np.ascontiguousarray(
        WO_w.reshape(8, 128, 8, 128).transpose(0, 2, 3, 1)).astype(bf)
    W1f = (W1 * ln1_g[None, :]).astype(np.float64)
    b1f = (b1 + W1 @ ln1_b).astype(f)
    w1tr = np.ascontiguousarray(
        W1f.reshape(32, 128, 8, 128).transpose(0, 2, 3, 1)).astype(f)
    w2tr = np.ascontiguousarray(
        W2.reshape(8, 128, 32, 128).transpose(0, 2, 3, 1)).astype(bf)
    wob_t = np.ascontiguousarray(WO_b.reshape(8, 128).T).astype(f)
    b1_t = np.ascontiguousarray(b1f.reshape(32, 128).T).astype(f)
    b2f = (b2 + ln1_b).astype(f)
    b2_t = np.ascontiguousarray(b2f.reshape(8, 128).T).astype(f)
    ln1g_t = np.ascontiguousarray(ln1_g.reshape(8, 128).T).astype(f)
    ln1b_t = np.ascontiguousarray(ln1_b.reshape(8, 128).T).astype(f)
    ln2g_t = np.ascontiguousarray(ln2_g.reshape(8, 128).T).astype(f)
    ln2b_t = np.ascontiguousarray(ln2_b.reshape(8, 128).T).astype(f)

    in_maps = []
    for c in range(NC):
        h0 = 2 * c
        # [2,DK,DM] -> [DM, 128]: W2h[j, hl*64+k] = W[h0+hl, k, j]
        wq2 = WQ_w[h0:h0 + 2].reshape(128, DM).T / 8.0
        wk2 = WK_w[h0:h0 + 2].reshape(128, DM).T
        wv2 = WV_w[h0:h0 + 2].reshape(128, DM).T
        # [8,128,128] layout: [ic, p, j] = W2h[ic*128+p, j]
        wqt = np.ascontiguousarray(wq2.reshape(8, 128, 128)).astype(bf)
        wkt = np.ascontiguousarray(wk2.reshape(8, 128, 128)).astype(bf)
        wvt = np.ascontiguousarray(wv2.reshape(8, 128, 128)).astype(bf)
        bq = WQ_b[h0:h0 + 2].reshape(128) / 8.0
        bk = WK_b[h0:h0 + 2].reshape(128)
        bv = WV_b[h0:h0 + 2].reshape(128)
        bqkv = np.stack([bq, bk, bv], axis=1).astype(f)
        in_maps.append({
            "xt": XT.astype(bf),
            "xts": np.ascontiguousarray(XT[:, :, c * SS:(c + 1) * SS]),
            "wqt": wqt, "wkt": wkt, "wvt": wvt, "bqkv": bqkv,
            "wotr": wotr, "wob": wob_t,
            "w1tr": w1tr, "b1": b1_t, "w2tr": w2tr, "b2": b2_t,
            "ln1g": ln1g_t, "ln1b": ln1b_t, "ln2g": ln2g_t, "ln2b": ln2b_t,
        })
    return in_maps


def run_in_maps(in_maps):
    """Run the compiled kernel on prepared in_maps; returns list of out dicts."""
    import jax
    r = _get_runner()
    n = NC
    per_core = [[np.asarray(m[name]) for name in r.in_names] for m in in_maps]
    concat_in = [
        np.concatenate([per_core[c][i] for c in range(n)], axis=0)
        for i in range(len(r.in_names))
    ]
    concat_zeros = [
        np.zeros((n * a.shape[0], *a.shape[1:]), a.dtype) for a in r.out_avals
    ]
    out_arrs = r.fn(*concat_in, *concat_zeros)
    out_arrs = [np.asarray(a) for a in out_arrs]
    return [
        {name: out_arrs[i].reshape(n, *r.out_avals[i].shape)[c]
         for i, name in enumerate(r.out_names)}
        for c in range(n)
    ]


def kernel(**inputs):
    in_maps = _prep_in_maps(**inputs)
    results = run_in_maps(in_maps)
    # assemble: each core's ot is [DM, B, SS] covering s in [c*SS,(c+1)*SS)
    ot_full = np.concatenate([results[c]["ot"] for c in range(NC)], axis=2)
    # [DM, B, S] -> [S, B, DM]
    return np.ascontiguousarray(ot_full.transpose(2, 1, 0))


# revision 15
# speedup vs baseline: 1.0857x; 1.0857x over previous
"""Trainium2 Bass kernel for nn_AttentionBlock (dense transformer block).

Strategy (8 NeuronCores, one chip):
  - Attention head-parallel: core c owns heads {2c, 2c+1} for all batches;
    computes Q/K/V projections (only its 2 heads), scores^T, softmax (exp on
    ScalarE, denominator via ones-column in the V matmul, DVE reciprocal),
    and the normalized context ctx^T.
  - AllToAll redistributes ctx^T shards so core c gets ALL heads for its
    S/8 = 256-query token shard.
  - Token-parallel back half: WO + residual + LN1 + MLP(relu) + residual +
    LN2 for the core's 1024 tokens (4 batches x 256 queries).
  All data flows feature-major ("transposed"): tiles are [feature_partition,
  token_free], so every matmul contraction runs on the partition dim.

Perf structure (v1):
  - Scores for head0 (partitions 0:64) and head1 (64:128) are emitted as
    adjacent matmuls -> PE row-tiling (tile_position (0,0)/(64,0)) runs
    them concurrently in the array: scores PE time halves.
  - Attention is sc-major; both heads' ctx chains trail the scores by one
    l-chunk; softmax normalization (reciprocal broadcast + stage multiply)
    is pipelined one sc-group behind so the PE never waits on the DVE
    reciprocal chain.
  - proj is double-buffered; QKV projection of batch b+1 interleaves into
    the exp-bound attention of batch b; V-transposes of b+1 run in the
    inter-batch gap; WO of back-half chunk0 interleaves into batch 3.
  - LayerNorm sum/sumsq chains are col-tiled into one PSUM tile (rows 0
    and 32 -> concurrent in the array); Square/ReLU run on DVE so ScalarE
    does exp (the attention pacer) only.
"""
import numpy as np

import concourse.bass as bass
import concourse.tile as tile
from concourse import mybir, bacc
from concourse.masks import make_identity

F32 = mybir.dt.float32
F32R = mybir.dt.float32r
BF16 = mybir.dt.bfloat16
AF = mybir.ActivationFunctionType
ALU = mybir.AluOpType

S, B, DM, H, DFF = 2048, 4, 1024, 16, 4096
DK = DM // H  # 64
NC = 8
SS = S // NC  # 256: seq shard per core
T = B * SS  # 1024 tokens per core in the back half
EPS = 1e-5

_CACHE = {}


def _build_nc():
    nc = bacc.Bacc("TRN2", target_bir_lowering=False, debug=False, num_devices=NC)

    # ---------------- I/O ----------------
    xt = nc.declare_dram_parameter("xt", [DM, B, S], BF16, isOutput=False)
    xts = nc.declare_dram_parameter("xts", [DM, B, SS], F32, isOutput=False)
    wqt = nc.declare_dram_parameter("wqt", [8, 128, 128], BF16, isOutput=False)
    wkt = nc.declare_dram_parameter("wkt", [8, 128, 128], BF16, isOutput=False)
    wvt = nc.declare_dram_parameter("wvt", [8, 128, 128], BF16, isOutput=False)
    bqkv = nc.declare_dram_parameter("bqkv", [128, 3], F32, isOutput=False)
    wotr = nc.declare_dram_parameter("wotr", [8, 8, 128, 128], BF16, isOutput=False)
    wob = nc.declare_dram_parameter("wob", [128, 8], F32, isOutput=False)
    w1tr = nc.declare_dram_parameter("w1tr", [32, 8, 128, 128], F32R, isOutput=False)
    b1 = nc.declare_dram_parameter("b1", [128, 32], F32, isOutput=False)
    w2tr = nc.declare_dram_parameter("w2tr", [8, 32, 128, 128], BF16, isOutput=False)
    b2 = nc.declare_dram_parameter("b2", [128, 8], F32, isOutput=False)
    ln1g = nc.declare_dram_parameter("ln1g", [128, 8], F32, isOutput=False)
    ln1b = nc.declare_dram_parameter("ln1b", [128, 8], F32, isOutput=False)
    ln2g = nc.declare_dram_parameter("ln2g", [128, 8], F32, isOutput=False)
    ln2b = nc.declare_dram_parameter("ln2b", [128, 8], F32, isOutput=False)
    ot = nc.declare_dram_parameter("ot", [DM, B, SS], F32, isOutput=True)

    with tile.TileContext(nc) as tc, nc.allow_low_precision(
        reason="float32r matmul operands (TF32-like) are intentional"
    ):
        with (
            tc.tile_pool(name="dram", bufs=1, space="DRAM") as dram,
            tc.tile_pool(name="const", bufs=1) as const,
            tc.tile_pool(name="wres", bufs=1) as wres,
            tc.tile_pool(name="qkv", bufs=1) as qkvp,
            tc.tile_pool(name="gen", bufs=2) as gen,
            tc.tile_pool(name="ps", bufs=2, space="PSUM") as ps,
        ):
            a2a_in = []
            a2a_out = []
            for b in range(B):
                ai = dram.tile([NC, 2, DK, SS], BF16, tag=f"a2a_in{b}",
                               name=f"a2a_in{b}")
                ao = dram.tile([NC, 2, DK, SS], BF16, tag=f"a2a_out{b}",
                               name=f"a2a_out{b}")
                a2a_in.append(ai)
                a2a_out.append(ao)

            # ---------------- constants ----------------
            ident = const.tile([128, 128], BF16, tag="ident")
            make_identity(nc, ident[:])
            ones_1x64 = const.tile([1, 64], BF16, tag="ones_1x64")
            nc.gpsimd.memset(ones_1x64[:], 1.0)
            ones_1x128 = const.tile([1, 128], F32R, tag="ones_1x128")
            scr_r = gen.tile([1, 128], F32, tag="stat", bufs=3,
                             name="ones_scr_r")
            nc.vector.memset(scr_r[:], 1.0)
            nc.vector.tensor_copy(out=ones_1x128[:], in_=scr_r[:])
            ones_128x1 = const.tile([128, 1], F32R, tag="ones_128x1")
            scr_c = gen.tile([128, 1], F32, tag="lnw", bufs=3,
                             name="ones_scr_c")
            nc.vector.memset(scr_c[:], 1.0)
            nc.vector.tensor_copy(out=ones_128x1[:], in_=scr_c[:])
            eps_sb = const.tile([1, 1], F32, tag="eps")
            nc.vector.memset(eps_sb[:], EPS)
            bqkv_sb = const.tile([128, 3], F32, tag="bqkv")
            nc.scalar.dma_start(out=bqkv_sb[:], in_=bqkv[:, :])
            wob_sb = const.tile([128, 8], F32, tag="wob")
            nc.scalar.dma_start(out=wob_sb[:], in_=wob[:, :])
            b1_sb = const.tile([128, 32], F32, tag="b1")
            nc.scalar.dma_start(out=b1_sb[:], in_=b1[:, :])
            b2_sb = const.tile([128, 8], F32, tag="b2")
            nc.scalar.dma_start(out=b2_sb[:], in_=b2[:, :])
            ln_sb = {}
            for name, h in (("ln1g", ln1g), ("ln1b", ln1b), ("ln2g", ln2g),
                            ("ln2b", ln2b)):
                t_ = const.tile([128, 8], F32, tag=name)
                nc.scalar.dma_start(out=t_[:], in_=h[:, :])
                ln_sb[name] = t_

            # resident QKV weights: [128p, 8ic, 128(2h dk)]
            w_sb = {}
            for name, h in (("wq", wqt), ("wk", wkt), ("wv", wvt)):
                t_ = wres.tile([128, 8, 128], BF16, tag=name)
                nc.sync.dma_start(
                    out=t_[:], in_=h[:, :, :].rearrange("ic p j -> p ic j")
                )
                w_sb[name] = t_
            # warm start: first attention rhs tiles, issued before anything
            warm_xt = []
            for ic in range(8):
                x_ = gen.tile([128, 512], BF16, tag="xtin", bufs=8,
                              name=f"warm_xt{ic}")
                nc.sync.dma_start(
                    out=x_[:], in_=xt[ic * 128:(ic + 1) * 128, 0, 0:512])
                warm_xt.append(x_)

            # early prefetch of back-half weight streams (no data deps)
            pre_wo = {}
            pre_w1 = {}
            pre_w2 = {}
            for oc in range(2):
                t_ = gen.tile([128, 8, 128], BF16, tag="wo_c", bufs=2,
                              name=f"pre_wo{oc}")
                nc.gpsimd.dma_start(
                    out=t_[:],
                    in_=wotr[oc, :, :, :].rearrange("ic p j -> p ic j"))
                pre_wo[oc] = t_
            for fc in range(3):
                t_ = gen.tile([128, 8, 128], F32R, tag="w1_c", bufs=3,
                              name=f"pre_w1{fc}")
                nc.gpsimd.dma_start(
                    out=t_[:],
                    in_=w1tr[fc, :, :, :].rearrange("ic p j -> p ic j"))
                pre_w1[fc] = t_
            for half in range(2):
                t_ = gen.tile([128, 16, 128], BF16, tag="w2_c", bufs=2,
                              name=f"pre_w2{half}")
                nc.gpsimd.dma_start(
                    out=t_[:],
                    in_=w2tr[0, half * 16:(half + 1) * 16, :, :]
                    .rearrange("fc p j -> p fc j"))
                pre_w2[half] = t_

            # persistent double-buffered per-batch state
            proj_sets = []
            for st in range(2):
                d = {}
                for name in ("wq", "wk", "wv"):
                    d[name] = qkvp.tile([128, S], BF16, tag=f"{name}{st}",
                                        name=f"{name}_s{st}")
                proj_sets.append(d)
            vt_sets = []
            for st in range(2):
                vts = []
                for lc in range(16):
                    v_ = const.tile([128, 130], BF16, tag=f"vt{st}_{lc}")
                    nc.gpsimd.memset(v_[:, 64:65], 1.0)
                    nc.gpsimd.memset(v_[:, 129:130], 1.0)
                    vts.append(v_)
                vt_sets.append(vts)

            # ---------- emission helpers ----------
            def load_xt(b, sc):
                if b == 0 and sc == 0:
                    return warm_xt
                xt_t = []
                for ic in range(8):
                    x_ = gen.tile([128, 512], BF16, tag="xtin", bufs=8)
                    nc.gpsimd.dma_start(
                        out=x_[:],
                        in_=xt[ic * 128:(ic + 1) * 128, b,
                               sc * 512:(sc + 1) * 512],
                    )
                    xt_t.append(x_)
                return xt_t

            def emit_p1_proj(b, sc, pi, name, xt_t):
                """One projection chain (wq/wk/wv) for batch b chunk sc."""
                proj = proj_sets[b % 2]
                pmm = ps.tile([128, 512], F32, tag="psc", bufs=3,
                              padded_shape=[128, 1024],
                              name=f"p1_{b}_{sc}_{name}")
                for ic in range(8):
                    nc.tensor.matmul(
                        out=pmm[:],
                        lhsT=w_sb[name][:, ic, :],
                        rhs=xt_t[ic][:],
                        start=(ic == 0),
                        stop=(ic == 7),
                    )
                nc.vector.tensor_scalar_add(
                    out=proj[name][:, sc * 512:(sc + 1) * 512],
                    in0=pmm[:],
                    scalar1=bqkv_sb[:, pi:pi + 1],
                )

            def emit_p2_chunk(b, lc):
                """Transpose one 128-token V chunk into vt tiles."""
                proj = proj_sets[b % 2]
                vts = vt_sets[b % 2]
                ptr = ps.tile([128, 128], BF16, tag="pb1", bufs=2,
                              padded_shape=[128, 512])
                nc.tensor.transpose(
                    out=ptr[:],
                    in_=proj["wv"][:, lc * 128:(lc + 1) * 128],
                    identity=ident[:],
                )
                for hl in range(2):
                    nc.vector.tensor_copy(
                        out=vts[lc][:, hl * 65:hl * 65 + 64],
                        in_=ptr[:, hl * 64:(hl + 1) * 64],
                    )

            def emit_softmax_norm(b, sc, pctx, recips):
                """Reciprocal broadcast + stage multiply + a2a shard DMAs
                for group (b, sc).  Called one group late so the PE's pbc
                matmul never waits on the DVE reciprocal chain."""
                for hl in range(2):
                    nc.tensor.matmul(
                        out=pctx[hl][64:128, :],
                        lhsT=ones_1x64[:],
                        rhs=recips[hl][:],
                        start=True,
                        stop=True,
                    )
                for hl in range(2):
                    bc = gen.tile([64, 512], BF16, tag="bc", bufs=1)
                    nc.vector.tensor_copy(out=bc[:], in_=pctx[hl][64:128, :])
                    stage = gen.tile([64, 512], BF16, tag="stage", bufs=2)
                    nc.vector.tensor_tensor(
                        out=stage[:], in0=pctx[hl][0:64, :], in1=bc[:],
                        op=ALU.mult,
                    )
                    for half in range(2):
                        d = sc * 2 + half
                        nc.sync.dma_start(
                            out=a2a_in[b][d, hl, :, :],
                            in_=stage[:, half * 256:(half + 1) * 256],
                        )

            def emit_attn_sc(b, sc, extra, pending):
                """Both heads' scores/exp/ctx for group (b, sc); returns
                the group's (pctx, recips) for deferred normalization.

                extra: dict lc2 -> list of emission thunks (PE filler).
                pending: previous group's deferred normalization (emitted
                at lc2==0, after this group's first score pair)."""
                proj = proj_sets[b % 2]
                vts = vt_sets[b % 2]
                exp_t = [[], []]
                pctx = [None, None]
                for lc2 in range(8):
                    psc = [None, None]
                    for hl in range(2):
                        psc[hl] = ps.tile([128, 1024], F32, tag="psc",
                                          bufs=3,
                                          name=f"psc{b}_{sc}_{lc2}_{hl}")
                    for k in range(2):
                        for hl in range(2):
                            hb = hl * 64
                            nc.tensor.matmul(
                                out=psc[hl][:, k * 512:(k + 1) * 512],
                                lhsT=proj["wk"][hb:hb + 64,
                                                (lc2 * 2 + k) * 128:
                                                (lc2 * 2 + k + 1) * 128],
                                rhs=proj["wq"][hb:hb + 64,
                                               sc * 512:(sc + 1) * 512],
                                start=True,
                                stop=True,
                            )
                    if lc2 == 0 and pending is not None:
                        emit_softmax_norm(*pending)
                    for hl in range(2):
                        e_ = gen.tile([128, 1024], BF16, tag="u2k", bufs=16,
                                      name=f"e{b}_{sc}_{lc2}_{hl}")
                        nc.scalar.activation(
                            out=e_[:], in_=psc[hl][:], func=AF.Exp
                        )
                        exp_t[hl].append(e_)
                    if lc2 == 1:
                        for hl in range(2):
                            pctx[hl] = ps.tile(
                                [128, 512], F32, tag="pb1", bufs=2,
                                name=f"pctx{b}_{sc}_{hl}")
                    if lc2 >= 1:
                        src = lc2 - 1
                        for hl in range(2):
                            for k in range(2):
                                lc = src * 2 + k
                                nc.tensor.matmul(
                                    out=pctx[hl][0:65, :],
                                    lhsT=vts[lc][:, hl * 65:hl * 65 + 65],
                                    rhs=exp_t[hl][src][:, k * 512:
                                                       (k + 1) * 512],
                                    start=(lc == 0),
                                    stop=False,
                                )
                    for th in extra.get(lc2, ()):
                        th()
                # finish both ctx chains (src=7)
                for hl in range(2):
                    for k in range(2):
                        lc = 14 + k
                        nc.tensor.matmul(
                            out=pctx[hl][0:65, :],
                            lhsT=vts[lc][:, hl * 65:hl * 65 + 65],
                            rhs=exp_t[hl][7][:, k * 512:(k + 1) * 512],
                            start=False,
                            stop=(lc == 15),
                        )
                # denominators -> reciprocals (DVE; consumed next group)
                recips = [None, None]
                for hl in range(2):
                    dsum = gen.tile([1, 512], F32, tag="dsum", bufs=1)
                    nc.vector.tensor_copy(out=dsum[:],
                                          in_=pctx[hl][64:65, :])
                    rf = gen.tile([1, 512], F32, tag="rf32", bufs=1)
                    nc.vector.reciprocal_approx_fast(out=rf[:], in_=dsum[:])
                    recips[hl] = gen.tile([1, 512], BF16, tag="recip",
                                          bufs=2, name=f"rcp{b}_{sc}_{hl}")
                    nc.vector.tensor_copy(out=recips[hl][:], in_=rf[:])
                return pctx, recips

            # =====================================================
            # Back-half emission helpers
            # =====================================================
            def ln_stats(rt_tiles, tagn):
                # sum (row 0) and sumsq (row 32) chains col-tiled into one
                # PSUM tile -> concurrent in the PE array
                psum_s = ps.tile([1, 512], F32, tag="pb1", bufs=2,
                                 padded_shape=[128, 512],
                                 name=f"psum_s_{tagn}")
                sq_t = []
                for ic in range(8):
                    sq = gen.tile([128, 512], F32R, tag="sq", bufs=2,
                                  name=f"sq_{tagn}_{ic}")
                    nc.vector.tensor_tensor(out=sq[:], in0=rt_tiles[ic][:],
                                            in1=rt_tiles[ic][:],
                                            op=ALU.mult)
                    sq_t.append(sq)
                for ic in range(8):
                    nc.tensor.matmul(
                        out=psum_s[:], lhsT=ones_128x1[:],
                        rhs=rt_tiles[ic][:],
                        start=(ic == 0), stop=(ic == 7),
                    )
                psum_q = ps.tile([1, 512], F32, tag="pb1", bufs=2,
                                 padded_shape=[128, 512],
                                 name=f"psum_q_{tagn}")
                for ic in range(8):
                    nc.tensor.matmul(
                        out=psum_q[:], lhsT=ones_128x1[:],
                        rhs=sq_t[ic][:],
                        start=(ic == 0), stop=(ic == 7),
                    )
                mu = gen.tile([1, 512], F32R, tag="mu", bufs=2,
                              name=f"mu_{tagn}")
                nc.vector.tensor_scalar_mul(out=mu[:], in0=psum_s[:],
                                            scalar1=1.0 / DM)
                ex2 = gen.tile([1, 512], F32, tag="stat", bufs=3,
                               name=f"ex2_{tagn}")
                nc.vector.tensor_scalar_mul(out=ex2[:], in0=psum_q[:],
                                            scalar1=1.0 / DM)
                musq = gen.tile([1, 512], F32, tag="stat", bufs=3,
                                name=f"musq_{tagn}")
                nc.vector.tensor_tensor(out=musq[:], in0=mu[:], in1=mu[:],
                                        op=ALU.mult)
                var = gen.tile([1, 512], F32, tag="stat", bufs=3,
                               name=f"var_{tagn}")
                nc.vector.tensor_tensor(out=var[:], in0=ex2[:], in1=musq[:],
                                        op=ALU.subtract)
                sd = gen.tile([1, 512], F32, tag="stat", bufs=3,
                              name=f"sd_{tagn}")
                nc.scalar.activation(out=sd[:], in_=var[:], func=AF.Sqrt,
                                     bias=eps_sb[:])
                rscr = gen.tile([1, 512], F32, tag="stat", bufs=3,
                                name=f"rscr_{tagn}")
                rf = gen.tile([1, 512], F32, tag="stat", bufs=3,
                              name=f"rf_{tagn}")
                nc.vector.reciprocal_approx_accurate(out=rf[:], in_=sd[:],
                                                     scratch=rscr[:])
                rstd = gen.tile([1, 512], F32R, tag="rstd", bufs=2,
                                name=f"rstd_{tagn}")
                nc.vector.tensor_copy(out=rstd[:], in_=rf[:])
                return mu, rstd

            def ln_norm(stats, rt_tiles, g_sb, b_sb, out_dtype, fold_gb,
                        tagn, out_cb=None):
                mu, rstd = stats
                pmu = ps.tile([128, 512], F32, tag="pb1", bufs=2,
                              name=f"pmu_{tagn}")
                nc.tensor.matmul(out=pmu[:], lhsT=ones_1x128[:], rhs=mu[:],
                                 start=True, stop=True)
                prs = ps.tile([128, 512], F32, tag="pb1", bufs=2,
                              name=f"prs_{tagn}")
                nc.tensor.matmul(out=prs[:], lhsT=ones_1x128[:], rhs=rstd[:],
                                 start=True, stop=True)
                outs = []
                for ic in range(8):
                    tmp = gen.tile([128, 512], F32, tag="lnw", bufs=3,
                                   name=f"tmp_{tagn}_{ic}")
                    nc.vector.tensor_tensor(out=tmp[:], in0=rt_tiles[ic][:],
                                            in1=pmu[:], op=ALU.subtract)
                    if fold_gb:
                        o_ = gen.tile([128, 512], out_dtype, tag="u2k",
                                      bufs=16, name=f"z_{tagn}_{ic}")
                        nc.vector.tensor_tensor(out=o_[:], in0=tmp[:],
                                                in1=prs[:], op=ALU.mult)
                    else:
                        a_ = gen.tile([128, 512], F32, tag="lnw", bufs=3,
                                      name=f"a_{tagn}_{ic}")
                        t2 = gen.tile([128, 512], F32, tag="lnw", bufs=3,
                                      name=f"t2_{tagn}_{ic}")
                        o_ = gen.tile([128, 512], out_dtype, tag="res8",
                                      bufs=4, name=f"o_{tagn}_{ic}")
                        if ic % 2 == 0:
                            nc.vector.tensor_scalar_mul(
                                out=a_[:], in0=prs[:],
                                scalar1=g_sb[:, ic:ic + 1])
                            nc.vector.tensor_tensor(out=t2[:], in0=tmp[:],
                                                    in1=a_[:], op=ALU.mult)
                            nc.vector.tensor_scalar_add(
                                out=o_[:], in0=t2[:],
                                scalar1=b_sb[:, ic:ic + 1])
                        else:
                            nc.scalar.activation(
                                out=a_[:], in_=prs[:], func=AF.Copy,
                                scale=g_sb[:, ic:ic + 1])
                            nc.vector.tensor_tensor(out=t2[:], in0=tmp[:],
                                                    in1=a_[:], op=ALU.mult)
                            nc.scalar.activation(
                                out=o_[:], in_=t2[:], func=AF.Identity,
                                bias=b_sb[:, ic:ic + 1])
                        if out_cb is not None:
                            out_cb(ic, o_)
                    outs.append(o_)
                return outs

            def emit_wo_oc(t2c, oc, ctx_t, r1_t):
                """One 128-feature output chunk of WO + residual."""
                if t2c == 0 and oc in pre_wo:
                    wo_c = pre_wo.pop(oc)
                else:
                    wo_c = gen.tile([128, 8, 128], BF16, tag="wo_c",
                                    bufs=2, name=f"wo_c_{t2c}_{oc}")
                    nc.sync.dma_start(
                        out=wo_c[:],
                        in_=wotr[oc, :, :, :].rearrange(
                            "ic p j -> p ic j"),
                    )
                pmm = ps.tile([128, 512], F32, tag="psc", bufs=3,
                              padded_shape=[128, 1024],
                              name=f"pwo_{t2c}_{oc}")
                for ic in range(8):
                    nc.tensor.matmul(
                        out=pmm[:], lhsT=wo_c[:, ic, :],
                        rhs=ctx_t[ic][:],
                        start=(ic == 0), stop=(ic == 7),
                    )
                x_ = gen.tile([128, 512], F32, tag="xres", bufs=2,
                              name=f"x_{t2c}_{oc}")
                b0 = t2c * 2
                nc.sync.dma_start(
                    out=x_[:],
                    in_=xts[oc * 128:(oc + 1) * 128,
                            b0:b0 + 2, :].rearrange("p b s -> p (b s)"),
                )
                r1 = gen.tile([128, 512], F32R, tag="r1t", bufs=9,
                              name=f"r1_{t2c}_{oc}")
                nc.vector.scalar_tensor_tensor(
                    out=r1[:], in0=pmm[:], scalar=wob_sb[:, oc:oc + 1],
                    in1=x_[:], op0=ALU.add, op1=ALU.add,
                )
                r1_t.append(r1)

            def load_ctx(t2c):
                b0 = t2c * 2
                ctx_t = []
                for ic in range(8):
                    c_ = gen.tile([128, 512], BF16, tag="ctx", bufs=8,
                                  name=f"c_{t2c}_{ic}")
                    for half in range(2):
                        nc.sync.dma_start(
                            out=c_[:, half * 256:(half + 1) * 256],
                            in_=a2a_out[b0 + half][ic, :, :, :].rearrange(
                                "hl v s -> (hl v) s"
                            ),
                        )
                    ctx_t.append(c_)
                return ctx_t

            def emit_mlp1(t2c, x1_t):
                h_t = []
                for fc in range(32):
                    if t2c == 0 and fc in pre_w1:
                        w1_c = pre_w1.pop(fc)
                    else:
                        w1_c = gen.tile([128, 8, 128], F32R, tag="w1_c",
                                        bufs=3, name=f"w1_c_{t2c}_{fc}")
                        nc.sync.dma_start(
                            out=w1_c[:],
                            in_=w1tr[fc, :, :, :].rearrange(
                                "ic p j -> p ic j"),
                        )
                    pmm = ps.tile([128, 512], F32, tag="psc", bufs=3,
                                  padded_shape=[128, 1024],
                                  name=f"ph_{t2c}_{fc}")
                    for ic in range(8):
                        nc.tensor.matmul(
                            out=pmm[:], lhsT=w1_c[:, ic, :], rhs=x1_t[ic][:],
                            start=(ic == 0), stop=(ic == 7),
                        )
                    h_ = gen.tile([128, 512], BF16, tag="ht", bufs=32,
                                  name=f"h_{t2c}_{fc}")
                    nc.vector.tensor_scalar(
                        out=h_[:], in0=pmm[:],
                        scalar1=b1_sb[:, fc:fc + 1], scalar2=0.0,
                        op0=ALU.add, op1=ALU.max,
                    )
                    h_t.append(h_)
                return h_t

            def emit_mlp2(t2c, h_t, x1_t):
                r2_t = []
                for oc in range(8):
                    pmm = ps.tile([128, 512], F32, tag="psc", bufs=3,
                                  padded_shape=[128, 1024],
                                  name=f"pm_{t2c}_{oc}")
                    for half in range(2):
                        if t2c == 0 and oc == 0 and half in pre_w2:
                            w2_c = pre_w2.pop(half)
                        else:
                            w2_c = gen.tile(
                                [128, 16, 128], BF16, tag="w2_c", bufs=2,
                                name=f"w2_c_{t2c}_{oc}_{half}")
                            nc.sync.dma_start(
                                out=w2_c[:],
                                in_=w2tr[oc, half * 16:(half + 1) * 16, :, :]
                                .rearrange("fc p j -> p fc j"),
                            )
                        for f2 in range(16):
                            fc = half * 16 + f2
                            nc.tensor.matmul(
                                out=pmm[:], lhsT=w2_c[:, f2, :],
                                rhs=h_t[fc][:],
                                start=(fc == 0), stop=(fc == 31),
                            )
                    r2p = gen.tile([128, 512], F32, tag="lnw", bufs=3,
                                   name=f"r2p_{t2c}_{oc}")
                    nc.vector.scalar_tensor_tensor(
                        out=r2p[:], in0=x1_t[oc][:],
                        scalar=ln_sb["ln1g"][:, oc:oc + 1],
                        in1=pmm[:], op0=ALU.mult, op1=ALU.add,
                    )
                    r2 = gen.tile([128, 512], F32R, tag="u2k", bufs=16,
                                  name=f"r2_{t2c}_{oc}")
                    nc.vector.tensor_scalar_add(
                        out=r2[:], in0=r2p[:], scalar1=b2_sb[:, oc:oc + 1],
                    )
                    r2_t.append(r2)
                return r2_t

            def emit_out(t2c, o_t):
                b0 = t2c * 2
                for oc in range(8):
                    nc.sync.dma_start(
                        out=ot[oc * 128:(oc + 1) * 128,
                               b0:b0 + 2, :].rearrange("p b s -> p (b s)"),
                        in_=o_t[oc][:],
                    )

            # =====================================================
            # Main schedule
            # =====================================================
            # Prologue: full QKV + V-transpose for batch 0
            for sc in range(4):
                xt_t = load_xt(0, sc)
                for pi, name in enumerate(("wq", "wk", "wv")):
                    emit_p1_proj(0, sc, pi, name, xt_t)
                for lc in range(4 * sc, 4 * sc + 4):
                    emit_p2_chunk(0, lc)

            ctx0_t = None
            r1_0 = []
            pending = None
            for b in range(B):
                for sc in range(4):
                    extra = {}
                    if b < 3:
                        # QKV of batch b+1, spread across this group
                        xt_box = []

                        def load_thunk(b=b, sc=sc, box=xt_box):
                            box.append(load_xt(b + 1, sc))

                        extra[0] = [load_thunk]
                        extra[1] = [lambda b=b, sc=sc, box=xt_box:
                                    emit_p1_proj(b + 1, sc, 0, "wq", box[0])]
                        extra[3] = [lambda b=b, sc=sc, box=xt_box:
                                    emit_p1_proj(b + 1, sc, 1, "wk", box[0])]
                        extra[5] = [lambda b=b, sc=sc, box=xt_box:
                                    emit_p1_proj(b + 1, sc, 2, "wv", box[0])]
                    else:
                        # WO of back-half chunk0 (a2a 0/1 done long ago)
                        extra[1] = [lambda sc=sc:
                                    emit_wo_oc(0, 2 * sc, ctx0_t, r1_0)]
                        extra[5] = [lambda sc=sc:
                                    emit_wo_oc(0, 2 * sc + 1, ctx0_t, r1_0)]
                    pctx, recips = emit_attn_sc(b, sc, extra, pending)
                    pending = (b, sc, pctx, recips)
                # flush last group's softmax before the a2a
                emit_softmax_norm(*pending)
                pending = None
                nc.gpsimd.collective_compute(
                    "AllToAll",
                    ALU.bypass,
                    replica_groups=[list(range(NC))],
                    ins=[a2a_in[b][:].opt()],
                    outs=[a2a_out[b][:].opt()],
                )
                if b < 3:
                    # V-transposes for batch b+1 in the inter-batch gap
                    for lc in range(16):
                        emit_p2_chunk(b + 1, lc)
                if b == 1:
                    # ctx loads for back-half chunk0 (consumed in batch 3)
                    ctx0_t = load_ctx(0)

            # Back half: chunk0 LN1+MLP covers a2a(b3) latency, then chunk1
            st1_0 = ln_stats(r1_0, "l1c0")
            x1_0 = ln_norm(st1_0, r1_0, None, None, F32R, True, "l1c0")
            h_0 = emit_mlp1(0, x1_0)
            ctx1_t = load_ctx(1)
            r1_1 = []
            for oc in range(8):
                emit_wo_oc(1, oc, ctx1_t, r1_1)
            st1_1 = ln_stats(r1_1, "l1c1")
            x1_1 = ln_norm(st1_1, r1_1, None, None, F32R, True, "l1c1")
            r2_0 = emit_mlp2(0, h_0, x1_0)
            st2_0 = ln_stats(r2_0, "l2c0")
            h_1 = emit_mlp1(1, x1_1)
            def out_cb0(ic, o_):
                nc.sync.dma_start(
                    out=ot[ic * 128:(ic + 1) * 128,
                           0:2, :].rearrange("p b s -> p (b s)"),
                    in_=o_[:],
                )
            ln_norm(st2_0, r2_0, ln_sb["ln2g"], ln_sb["ln2b"], F32,
                    False, "l2c0", out_cb=out_cb0)
            r2_1 = emit_mlp2(1, h_1, x1_1)
            st2_1 = ln_stats(r2_1, "l2c1")
            def out_cb1(ic, o_):
                nc.sync.dma_start(
                    out=ot[ic * 128:(ic + 1) * 128,
                           2:4, :].rearrange("p b s -> p (b s)"),
                    in_=o_[:],
                )
            ln_norm(st2_1, r2_1, ln_sb["ln2g"], ln_sb["ln2b"], F32,
                    False, "l2c1", out_cb=out_cb1)

    nc.compile()
    return nc


# ------------------------------------------------------------------
# Host side
# ------------------------------------------------------------------
def _get_runner():
    if "runner" in _CACHE:
        return _CACHE["runner"]
    import jax
    from jax.sharding import Mesh, PartitionSpec
    try:
        from jax.experimental.shard_map import shard_map
    except ImportError:
        from jax.shard_map import shard_map
    from concourse import bass2jax
    from concourse.bass2jax import _bass_exec_p, install_neuronx_cc_hook

    nc = _build_nc()
    install_neuronx_cc_hook()
    partition_name = nc.partition_id_tensor.name if nc.partition_id_tensor else None
    in_names, out_names, out_avals, zero_outs = [], [], [], []
    for alloc in nc.m.functions[0].allocations:
        if not isinstance(alloc, mybir.MemoryLocationSet):
            continue
        name = alloc.memorylocations[0].name
        if alloc.kind == "ExternalInput":
            if name != partition_name:
                in_names.append(name)
        elif alloc.kind == "ExternalOutput":
            out_names.append(name)
            shape = tuple(alloc.tensor_shape)
            dtype = mybir.dt.np(alloc.dtype)
            out_avals.append(jax.core.ShapedArray(shape, dtype))
            zero_outs.append(np.zeros(shape, dtype))
    n_params = len(in_names)
    all_in_names = list(in_names) + list(out_names)
    if partition_name is not None:
        all_in_names.append(partition_name)

    def _body(*args):
        operands = list(args)
        if partition_name is not None:
            operands.append(bass2jax.partition_id_tensor())
        outs = _bass_exec_p.bind(
            *operands,
            out_avals=tuple(out_avals),
            in_names=tuple(all_in_names),
            out_names=tuple(out_names),
            lowering_input_output_aliases=(),
            sim_require_finite=True,
            sim_require_nnan=True,
            nc=nc,
        )
        return tuple(outs)

    donate = tuple(range(n_params, n_params + len(out_names)))
    devices = jax.devices()[:NC]
    mesh = Mesh(np.asarray(devices), ("core",))
    in_specs = (PartitionSpec("core"),) * (n_params + len(out_names))
    out_specs = (PartitionSpec("core"),) * len(out_names)
    fn = jax.jit(
        shard_map(_body, mesh=mesh, in_specs=in_specs, out_specs=out_specs,
                  check_rep=False),
        donate_argnums=donate, keep_unused=True,
    )

    class R:
        pass

    r = R()
    r.fn = fn
    r.in_names = in_names
    r.out_names = out_names
    r.out_avals = out_avals
    _CACHE["runner"] = r
    return r


def _prep_in_maps(X, WQ_w, WQ_b, WK_w, WK_b, WV_w, WV_b, WO_w, WO_b,
                  ln1_g, ln1_b, W1, b1, W2, b2, ln2_g, ln2_b):
    import ml_dtypes
    f = np.float32
    bf = ml_dtypes.bfloat16
    XT = np.ascontiguousarray(X.transpose(2, 1, 0)).astype(f)  # [DM,B,S]
    wotr = np.ascontiguousarray(
        WO_w.reshape(8, 128, 8, 128).transpose(0, 2, 3, 1)).astype(bf)
    W1f = (W1 * ln1_g[None, :]).astype(np.float64)
    b1f = (b1 + W1 @ ln1_b).astype(f)
    w1tr = np.ascontiguousarray(
        W1f.reshape(32, 128, 8, 128).transpose(0, 2, 3, 1)).astype(f)
    w2tr = np.ascontiguousarray(
        W2.reshape(8, 128, 32, 128).transpose(0, 2, 3, 1)).astype(bf)
    wob_t = np.ascontiguousarray(WO_b.reshape(8, 128).T).astype(f)
    b1_t = np.ascontiguousarray(b1f.reshape(32, 128).T).astype(f)
    b2f = (b2 + ln1_b).astype(f)
    b2_t = np.ascontiguousarray(b2f.reshape(8, 128).T).astype(f)
    ln1g_t = np.ascontiguousarray(ln1_g.reshape(8, 128).T).astype(f)
    ln1b_t = np.ascontiguousarray(ln1_b.reshape(8, 128).T).astype(f)
    ln2g_t = np.ascontiguousarray(ln2_g.reshape(8, 128).T).astype(f)
    ln2b_t = np.ascontiguousarray(ln2_b.reshape(8, 128).T).astype(f)

    in_maps = []
    for c in range(NC):
        h0 = 2 * c
        # [2,DK,DM] -> [DM, 128]: W2h[j, hl*64+k] = W[h0+hl, k, j]
        wq2 = WQ_w[h0:h0 + 2].reshape(128, DM).T / 8.0
        wk2 = WK_w[h0:h0 + 2].reshape(128, DM).T
        wv2 = WV_w[h0:h0 + 2].reshape(128, DM).T
        # [8,128,128] layout: [ic, p, j] = W2h[ic*128+p, j]
        wqt = np.ascontiguousarray(wq2.reshape(8, 128, 128)).astype(bf)
        wkt = np.ascontiguousarray(wk2.reshape(8, 128, 128)).astype(bf)
        wvt = np.ascontiguousarray(wv2.reshape(8, 128, 128)).astype(bf)
        bq = WQ_b[h0:h0 + 2].reshape(128) / 8.0
        bk = WK_b[h0:h0 + 2].reshape(128)
        bv = WV_b[h0:h0 + 2].reshape(128)
        bqkv = np.stack([bq, bk, bv], axis=1).astype(f)
        in_maps.append({
            "xt": XT.astype(bf),
            "xts": np.ascontiguousarray(XT[:, :, c * SS:(c + 1) * SS]),
            "wqt": wqt, "wkt": wkt, "wvt": wvt, "bqkv": bqkv,
            "wotr": wotr, "wob": wob_t,
            "w1tr": w1tr, "b1": b1_t, "w2tr": w2tr, "b2": b2_t,
            "ln1g": ln1g_t, "ln1b": ln1b_t, "ln2g": ln2g_t, "ln2b": ln2b_t,
        })
    return in_maps


def run_in_maps(in_maps):
    """Run the compiled kernel on prepared in_maps; returns list of out dicts."""
    import jax
    r = _get_runner()
    n = NC
    per_core = [[np.asarray(m[name]) for name in r.in_names] for m in in_maps]
    concat_in = [
        np.concatenate([per_core[c][i] for c in range(n)], axis=0)
        for i in range(len(r.in_names))
    ]
    concat_zeros = [
        np.zeros((n * a.shape[0], *a.shape[1:]), a.dtype) for a in r.out_avals
    ]
    out_arrs = r.fn(*concat_in, *concat_zeros)
    out_arrs = [np.asarray(a) for a in out_arrs]
    return [
        {name: out_arrs[i].reshape(n, *r.out_avals[i].shape)[c]
         for i, name in enumerate(r.out_names)}
        for c in range(n)
    ]


def kernel(**inputs):
    in_maps = _prep_in_maps(**inputs)
    results = run_in_maps(in_maps)
    # assemble: each core's ot is [DM, B, SS] covering s in [c*SS,(c+1)*SS)
    ot_full = np.concatenate([results[c]["ot"] for c in range(NC)], axis=2)
    # [DM, B, S] -> [S, B, DM]
    return np.ascontiguousarray(ot_full.transpose(2, 1, 0))


# revision 16
# speedup vs baseline: 1.1285x; 1.0394x over previous
"""Trainium2 Bass kernel for nn_AttentionBlock (dense transformer block).

Strategy (8 NeuronCores, one chip):
  - Attention head-parallel: core c owns heads {2c, 2c+1} for all batches;
    computes Q/K/V projections (only its 2 heads), scores^T, softmax (exp on
    ScalarE, denominator via ones-column in the V matmul, DVE reciprocal),
    and the normalized context ctx^T.
  - AllToAll redistributes ctx^T shards so core c gets ALL heads for its
    S/8 = 256-query token shard.
  - Token-parallel back half: WO + residual + LN1 + MLP(relu) + residual +
    LN2 for the core's 1024 tokens (4 batches x 256 queries).
  All data flows feature-major ("transposed"): tiles are [feature_partition,
  token_free], so every matmul contraction runs on the partition dim.

Perf structure (v1):
  - Scores for head0 (partitions 0:64) and head1 (64:128) are emitted as
    adjacent matmuls -> PE row-tiling (tile_position (0,0)/(64,0)) runs
    them concurrently in the array: scores PE time halves.
  - Attention is sc-major; both heads' ctx chains trail the scores by one
    l-chunk; softmax normalization (reciprocal broadcast + stage multiply)
    is pipelined one sc-group behind so the PE never waits on the DVE
    reciprocal chain.
  - proj is double-buffered; QKV projection of batch b+1 interleaves into
    the exp-bound attention of batch b; V-transposes of b+1 run in the
    inter-batch gap; WO of back-half chunk0 interleaves into batch 3.
  - LayerNorm sum/sumsq chains are col-tiled into one PSUM tile (rows 0
    and 32 -> concurrent in the array); Square/ReLU run on DVE so ScalarE
    does exp (the attention pacer) only.
"""
import numpy as np

import concourse.bass as bass
import concourse.tile as tile
from concourse import mybir, bacc
from concourse.masks import make_identity

F32 = mybir.dt.float32
F32R = mybir.dt.float32r
BF16 = mybir.dt.bfloat16
AF = mybir.ActivationFunctionType
ALU = mybir.AluOpType

S, B, DM, H, DFF = 2048, 4, 1024, 16, 4096
DK = DM // H  # 64
NC = 8
SS = S // NC  # 256: seq shard per core
T = B * SS  # 1024 tokens per core in the back half
EPS = 1e-5

_CACHE = {}


def _build_nc():
    nc = bacc.Bacc("TRN2", target_bir_lowering=False, debug=False, num_devices=NC)

    # ---------------- I/O ----------------
    xt = nc.declare_dram_parameter("xt", [DM, B, S], BF16, isOutput=False)
    xts = nc.declare_dram_parameter("xts", [DM, B, SS], F32, isOutput=False)
    wqt = nc.declare_dram_parameter("wqt", [8, 128, 128], BF16, isOutput=False)
    wkt = nc.declare_dram_parameter("wkt", [8, 128, 128], BF16, isOutput=False)
    wvt = nc.declare_dram_parameter("wvt", [8, 128, 128], BF16, isOutput=False)
    bqkv = nc.declare_dram_parameter("bqkv", [128, 3], F32, isOutput=False)
    wotr = nc.declare_dram_parameter("wotr", [8, 8, 128, 128], BF16, isOutput=False)
    wob = nc.declare_dram_parameter("wob", [128, 8], F32, isOutput=False)
    w1tr = nc.declare_dram_parameter("w1tr", [32, 8, 128, 128], BF16, isOutput=False)
    b1 = nc.declare_dram_parameter("b1", [128, 32], F32, isOutput=False)
    w2tr = nc.declare_dram_parameter("w2tr", [8, 32, 128, 128], BF16, isOutput=False)
    b2 = nc.declare_dram_parameter("b2", [128, 8], F32, isOutput=False)
    ln1g = nc.declare_dram_parameter("ln1g", [128, 8], F32, isOutput=False)
    ln1b = nc.declare_dram_parameter("ln1b", [128, 8], F32, isOutput=False)
    ln2g = nc.declare_dram_parameter("ln2g", [128, 8], F32, isOutput=False)
    ln2b = nc.declare_dram_parameter("ln2b", [128, 8], F32, isOutput=False)
    ot = nc.declare_dram_parameter("ot", [DM, B, SS], F32, isOutput=True)

    with tile.TileContext(nc) as tc, nc.allow_low_precision(
        reason="float32r matmul operands (TF32-like) are intentional"
    ):
        with (
            tc.tile_pool(name="dram", bufs=1, space="DRAM") as dram,
            tc.tile_pool(name="const", bufs=1) as const,
            tc.tile_pool(name="wres", bufs=1) as wres,
            tc.tile_pool(name="qkv", bufs=1) as qkvp,
            tc.tile_pool(name="gen", bufs=2) as gen,
            tc.tile_pool(name="ps", bufs=2, space="PSUM") as ps,
        ):
            a2a_in = []
            a2a_out = []
            for b in range(B):
                ai = dram.tile([NC, 2, DK, SS], BF16, tag=f"a2a_in{b}",
                               name=f"a2a_in{b}")
                ao = dram.tile([NC, 2, DK, SS], BF16, tag=f"a2a_out{b}",
                               name=f"a2a_out{b}")
                a2a_in.append(ai)
                a2a_out.append(ao)

            # ---------------- constants ----------------
            ident = const.tile([128, 128], BF16, tag="ident")
            make_identity(nc, ident[:])
            ones_1x64 = const.tile([1, 64], BF16, tag="ones_1x64")
            nc.gpsimd.memset(ones_1x64[:], 1.0)
            ones_1x128 = const.tile([1, 128], F32R, tag="ones_1x128")
            scr_r = gen.tile([1, 128], F32, tag="stat", bufs=3,
                             name="ones_scr_r")
            nc.vector.memset(scr_r[:], 1.0)
            nc.vector.tensor_copy(out=ones_1x128[:], in_=scr_r[:])
            ones_128x1 = const.tile([128, 1], F32R, tag="ones_128x1")
            scr_c = gen.tile([128, 1], F32, tag="lnw", bufs=3,
                             name="ones_scr_c")
            nc.vector.memset(scr_c[:], 1.0)
            nc.vector.tensor_copy(out=ones_128x1[:], in_=scr_c[:])
            eps_sb = const.tile([1, 1], F32, tag="eps")
            nc.vector.memset(eps_sb[:], EPS)
            bqkv_sb = const.tile([128, 3], F32, tag="bqkv")
            nc.scalar.dma_start(out=bqkv_sb[:], in_=bqkv[:, :])
            wob_sb = const.tile([128, 8], F32, tag="wob")
            nc.scalar.dma_start(out=wob_sb[:], in_=wob[:, :])
            b1_sb = const.tile([128, 32], F32, tag="b1")
            nc.scalar.dma_start(out=b1_sb[:], in_=b1[:, :])
            b2_sb = const.tile([128, 8], F32, tag="b2")
            nc.scalar.dma_start(out=b2_sb[:], in_=b2[:, :])
            ln_sb = {}
            for name, h in (("ln1g", ln1g), ("ln1b", ln1b), ("ln2g", ln2g),
                            ("ln2b", ln2b)):
                t_ = const.tile([128, 8], F32, tag=name)
                nc.scalar.dma_start(out=t_[:], in_=h[:, :])
                ln_sb[name] = t_

            # resident QKV weights: [128p, 8ic, 128(2h dk)]
            w_sb = {}
            for name, h in (("wq", wqt), ("wk", wkt), ("wv", wvt)):
                t_ = wres.tile([128, 8, 128], BF16, tag=name)
                nc.sync.dma_start(
                    out=t_[:], in_=h[:, :, :].rearrange("ic p j -> p ic j")
                )
                w_sb[name] = t_
            # warm start: first attention rhs tiles, issued before anything
            warm_xt = []
            for ic in range(8):
                x_ = gen.tile([128, 512], BF16, tag="xtin", bufs=8,
                              name=f"warm_xt{ic}")
                nc.sync.dma_start(
                    out=x_[:], in_=xt[ic * 128:(ic + 1) * 128, 0, 0:512])
                warm_xt.append(x_)

            # early prefetch of back-half weight streams (no data deps)
            pre_wo = {}
            pre_w1 = {}
            pre_w2 = {}
            for oc in range(2):
                t_ = gen.tile([128, 8, 128], BF16, tag="wo_c", bufs=2,
                              name=f"pre_wo{oc}")
                nc.gpsimd.dma_start(
                    out=t_[:],
                    in_=wotr[oc, :, :, :].rearrange("ic p j -> p ic j"))
                pre_wo[oc] = t_
            for fc in range(3):
                t_ = gen.tile([128, 8, 128], BF16, tag="w1_c", bufs=3,
                              name=f"pre_w1{fc}")
                nc.gpsimd.dma_start(
                    out=t_[:],
                    in_=w1tr[fc, :, :, :].rearrange("ic p j -> p ic j"))
                pre_w1[fc] = t_
            for half in range(2):
                t_ = gen.tile([128, 16, 128], BF16, tag="w2_c", bufs=2,
                              name=f"pre_w2{half}")
                nc.gpsimd.dma_start(
                    out=t_[:],
                    in_=w2tr[0, half * 16:(half + 1) * 16, :, :]
                    .rearrange("fc p j -> p fc j"))
                pre_w2[half] = t_

            # persistent double-buffered per-batch state
            proj_sets = []
            for st in range(2):
                d = {}
                for name in ("wq", "wk", "wv"):
                    d[name] = qkvp.tile([128, S], BF16, tag=f"{name}{st}",
                                        name=f"{name}_s{st}")
                proj_sets.append(d)
            vt_sets = []
            for st in range(2):
                vts = []
                for lc in range(16):
                    v_ = const.tile([128, 130], BF16, tag=f"vt{st}_{lc}")
                    nc.gpsimd.memset(v_[:, 64:65], 1.0)
                    nc.gpsimd.memset(v_[:, 129:130], 1.0)
                    vts.append(v_)
                vt_sets.append(vts)

            # ---------- emission helpers ----------
            def load_xt(b, sc):
                if b == 0 and sc == 0:
                    return warm_xt
                xt_t = []
                for ic in range(8):
                    x_ = gen.tile([128, 512], BF16, tag="xtin", bufs=8)
                    nc.gpsimd.dma_start(
                        out=x_[:],
                        in_=xt[ic * 128:(ic + 1) * 128, b,
                               sc * 512:(sc + 1) * 512],
                    )
                    xt_t.append(x_)
                return xt_t

            def emit_p1_proj(b, sc, pi, name, xt_t):
                """One projection chain (wq/wk/wv) for batch b chunk sc."""
                proj = proj_sets[b % 2]
                pmm = ps.tile([128, 512], F32, tag="psc", bufs=3,
                              padded_shape=[128, 1024],
                              name=f"p1_{b}_{sc}_{name}")
                for ic in range(8):
                    nc.tensor.matmul(
                        out=pmm[:],
                        lhsT=w_sb[name][:, ic, :],
                        rhs=xt_t[ic][:],
                        start=(ic == 0),
                        stop=(ic == 7),
                    )
                nc.vector.tensor_scalar_add(
                    out=proj[name][:, sc * 512:(sc + 1) * 512],
                    in0=pmm[:],
                    scalar1=bqkv_sb[:, pi:pi + 1],
                )

            def emit_p2_chunk(b, lc):
                """Transpose one 128-token V chunk into vt tiles."""
                proj = proj_sets[b % 2]
                vts = vt_sets[b % 2]
                ptr = ps.tile([128, 128], BF16, tag="pb1", bufs=2,
                              padded_shape=[128, 512])
                nc.tensor.transpose(
                    out=ptr[:],
                    in_=proj["wv"][:, lc * 128:(lc + 1) * 128],
                    identity=ident[:],
                )
                for hl in range(2):
                    nc.vector.tensor_copy(
                        out=vts[lc][:, hl * 65:hl * 65 + 64],
                        in_=ptr[:, hl * 64:(hl + 1) * 64],
                    )

            def emit_softmax_norm(b, sc, pctx, recips):
                """Reciprocal broadcast + stage multiply + a2a shard DMAs
                for group (b, sc).  Called one group late so the PE's pbc
                matmul never waits on the DVE reciprocal chain."""
                for hl in range(2):
                    nc.tensor.matmul(
                        out=pctx[hl][64:128, :],
                        lhsT=ones_1x64[:],
                        rhs=recips[hl][:],
                        start=True,
                        stop=True,
                    )
                for hl in range(2):
                    bc = gen.tile([64, 512], BF16, tag="bc", bufs=1)
                    nc.vector.tensor_copy(out=bc[:], in_=pctx[hl][64:128, :])
                    stage = gen.tile([64, 512], BF16, tag="stage", bufs=2)
                    nc.vector.tensor_tensor(
                        out=stage[:], in0=pctx[hl][0:64, :], in1=bc[:],
                        op=ALU.mult,
                    )
                    for half in range(2):
                        d = sc * 2 + half
                        nc.sync.dma_start(
                            out=a2a_in[b][d, hl, :, :],
                            in_=stage[:, half * 256:(half + 1) * 256],
                        )

            def emit_attn_sc(b, sc, extra, pending):
                """Both heads' scores/exp/ctx for group (b, sc); returns
                the group's (pctx, recips) for deferred normalization.

                extra: dict lc2 -> list of emission thunks (PE filler).
                pending: previous group's deferred normalization (emitted
                at lc2==0, after this group's first score pair)."""
                proj = proj_sets[b % 2]
                vts = vt_sets[b % 2]
                exp_t = [[], []]
                pctx = [None, None]
                for lc2 in range(8):
                    psc = [None, None]
                    for hl in range(2):
                        psc[hl] = ps.tile([128, 1024], F32, tag="psc",
                                          bufs=3,
                                          name=f"psc{b}_{sc}_{lc2}_{hl}")
                    for k in range(2):
                        for hl in range(2):
                            hb = hl * 64
                            nc.tensor.matmul(
                                out=psc[hl][:, k * 512:(k + 1) * 512],
                                lhsT=proj["wk"][hb:hb + 64,
                                                (lc2 * 2 + k) * 128:
                                                (lc2 * 2 + k + 1) * 128],
                                rhs=proj["wq"][hb:hb + 64,
                                               sc * 512:(sc + 1) * 512],
                                start=True,
                                stop=True,
                            )
                    if lc2 == 0 and pending is not None:
                        emit_softmax_norm(*pending)
                    for hl in range(2):
                        e_ = gen.tile([128, 1024], BF16, tag="u2k", bufs=16,
                                      name=f"e{b}_{sc}_{lc2}_{hl}")
                        nc.scalar.activation(
                            out=e_[:], in_=psc[hl][:], func=AF.Exp
                        )
                        exp_t[hl].append(e_)
                    if lc2 == 1:
                        for hl in range(2):
                            pctx[hl] = ps.tile(
                                [128, 512], F32, tag="pb1", bufs=2,
                                name=f"pctx{b}_{sc}_{hl}")
                    if lc2 >= 1:
                        src = lc2 - 1
                        for hl in range(2):
                            for k in range(2):
                                lc = src * 2 + k
                                nc.tensor.matmul(
                                    out=pctx[hl][0:65, :],
                                    lhsT=vts[lc][:, hl * 65:hl * 65 + 65],
                                    rhs=exp_t[hl][src][:, k * 512:
                                                       (k + 1) * 512],
                                    start=(lc == 0),
                                    stop=False,
                                )
                    for th in extra.get(lc2, ()):
                        th()
                # finish both ctx chains (src=7)
                for hl in range(2):
                    for k in range(2):
                        lc = 14 + k
                        nc.tensor.matmul(
                            out=pctx[hl][0:65, :],
                            lhsT=vts[lc][:, hl * 65:hl * 65 + 65],
                            rhs=exp_t[hl][7][:, k * 512:(k + 1) * 512],
                            start=False,
                            stop=(lc == 15),
                        )
                # denominators -> reciprocals (DVE; consumed next group)
                recips = [None, None]
                for hl in range(2):
                    dsum = gen.tile([1, 512], F32, tag="dsum", bufs=1)
                    nc.vector.tensor_copy(out=dsum[:],
                                          in_=pctx[hl][64:65, :])
                    rf = gen.tile([1, 512], F32, tag="rf32", bufs=1)
                    nc.vector.reciprocal_approx_fast(out=rf[:], in_=dsum[:])
                    recips[hl] = gen.tile([1, 512], BF16, tag="recip",
                                          bufs=2, name=f"rcp{b}_{sc}_{hl}")
                    nc.vector.tensor_copy(out=recips[hl][:], in_=rf[:])
                return pctx, recips

            # =====================================================
            # Back-half emission helpers
            # =====================================================
            def ln_stats(rt_tiles, tagn):
                # sum (row 0) and sumsq (row 32) chains col-tiled into one
                # PSUM tile -> concurrent in the PE array
                psum_s = ps.tile([1, 512], F32, tag="pb1", bufs=2,
                                 padded_shape=[128, 512],
                                 name=f"psum_s_{tagn}")
                sq_t = []
                for ic in range(8):
                    sq = gen.tile([128, 512], F32R, tag="sq", bufs=2,
                                  name=f"sq_{tagn}_{ic}")
                    nc.vector.tensor_tensor(out=sq[:], in0=rt_tiles[ic][:],
                                            in1=rt_tiles[ic][:],
                                            op=ALU.mult)
                    sq_t.append(sq)
                for ic in range(8):
                    nc.tensor.matmul(
                        out=psum_s[:], lhsT=ones_128x1[:],
                        rhs=rt_tiles[ic][:],
                        start=(ic == 0), stop=(ic == 7),
                    )
                psum_q = ps.tile([1, 512], F32, tag="pb1", bufs=2,
                                 padded_shape=[128, 512],
                                 name=f"psum_q_{tagn}")
                for ic in range(8):
                    nc.tensor.matmul(
                        out=psum_q[:], lhsT=ones_128x1[:],
                        rhs=sq_t[ic][:],
                        start=(ic == 0), stop=(ic == 7),
                    )
                mu = gen.tile([1, 512], F32R, tag="mu", bufs=2,
                              name=f"mu_{tagn}")
                nc.vector.tensor_scalar_mul(out=mu[:], in0=psum_s[:],
                                            scalar1=1.0 / DM)
                ex2 = gen.tile([1, 512], F32, tag="stat", bufs=3,
                               name=f"ex2_{tagn}")
                nc.vector.tensor_scalar_mul(out=ex2[:], in0=psum_q[:],
                                            scalar1=1.0 / DM)
                musq = gen.tile([1, 512], F32, tag="stat", bufs=3,
                                name=f"musq_{tagn}")
                nc.vector.tensor_tensor(out=musq[:], in0=mu[:], in1=mu[:],
                                        op=ALU.mult)
                var = gen.tile([1, 512], F32, tag="stat", bufs=3,
                               name=f"var_{tagn}")
                nc.vector.tensor_tensor(out=var[:], in0=ex2[:], in1=musq[:],
                                        op=ALU.subtract)
                sd = gen.tile([1, 512], F32, tag="stat", bufs=3,
                              name=f"sd_{tagn}")
                nc.scalar.activation(out=sd[:], in_=var[:], func=AF.Sqrt,
                                     bias=eps_sb[:])
                rscr = gen.tile([1, 512], F32, tag="stat", bufs=3,
                                name=f"rscr_{tagn}")
                rf = gen.tile([1, 512], F32, tag="stat", bufs=3,
                              name=f"rf_{tagn}")
                nc.vector.reciprocal_approx_accurate(out=rf[:], in_=sd[:],
                                                     scratch=rscr[:])
                rstd = gen.tile([1, 512], F32R, tag="rstd", bufs=2,
                                name=f"rstd_{tagn}")
                nc.vector.tensor_copy(out=rstd[:], in_=rf[:])
                return mu, rstd

            def ln_norm(stats, rt_tiles, g_sb, b_sb, out_dtype, fold_gb,
                        tagn, out_cb=None):
                mu, rstd = stats
                pmu = ps.tile([128, 512], F32, tag="pb1", bufs=2,
                              name=f"pmu_{tagn}")
                nc.tensor.matmul(out=pmu[:], lhsT=ones_1x128[:], rhs=mu[:],
                                 start=True, stop=True)
                prs = ps.tile([128, 512], F32, tag="pb1", bufs=2,
                              name=f"prs_{tagn}")
                nc.tensor.matmul(out=prs[:], lhsT=ones_1x128[:], rhs=rstd[:],
                                 start=True, stop=True)
                outs = []
                for ic in range(8):
                    tmp = gen.tile([128, 512], F32, tag="lnw", bufs=3,
                                   name=f"tmp_{tagn}_{ic}")
                    nc.vector.tensor_tensor(out=tmp[:], in0=rt_tiles[ic][:],
                                            in1=pmu[:], op=ALU.subtract)
                    if fold_gb:
                        o_ = gen.tile([128, 512], out_dtype, tag="u2k",
                                      bufs=16, name=f"z_{tagn}_{ic}")
                        nc.vector.tensor_tensor(out=o_[:], in0=tmp[:],
                                                in1=prs[:], op=ALU.mult)
                    else:
                        a_ = gen.tile([128, 512], F32, tag="lnw", bufs=3,
                                      name=f"a_{tagn}_{ic}")
                        t2 = gen.tile([128, 512], F32, tag="lnw", bufs=3,
                                      name=f"t2_{tagn}_{ic}")
                        o_ = gen.tile([128, 512], out_dtype, tag="res8",
                                      bufs=4, name=f"o_{tagn}_{ic}")
                        if ic % 2 == 0:
                            nc.vector.tensor_scalar_mul(
                                out=a_[:], in0=prs[:],
                                scalar1=g_sb[:, ic:ic + 1])
                            nc.vector.tensor_tensor(out=t2[:], in0=tmp[:],
                                                    in1=a_[:], op=ALU.mult)
                            nc.vector.tensor_scalar_add(
                                out=o_[:], in0=t2[:],
                                scalar1=b_sb[:, ic:ic + 1])
                        else:
                            nc.scalar.activation(
                                out=a_[:], in_=prs[:], func=AF.Copy,
                                scale=g_sb[:, ic:ic + 1])
                            nc.vector.tensor_tensor(out=t2[:], in0=tmp[:],
                                                    in1=a_[:], op=ALU.mult)
                            nc.scalar.activation(
                                out=o_[:], in_=t2[:], func=AF.Identity,
                                bias=b_sb[:, ic:ic + 1])
                        if out_cb is not None:
                            out_cb(ic, o_)
                    outs.append(o_)
                return outs

            def emit_wo_oc(t2c, oc, ctx_t, r1_t):
                """One 128-feature output chunk of WO + residual."""
                if t2c == 0 and oc in pre_wo:
                    wo_c = pre_wo.pop(oc)
                else:
                    wo_c = gen.tile([128, 8, 128], BF16, tag="wo_c",
                                    bufs=2, name=f"wo_c_{t2c}_{oc}")
                    nc.sync.dma_start(
                        out=wo_c[:],
                        in_=wotr[oc, :, :, :].rearrange(
                            "ic p j -> p ic j"),
                    )
                pmm = ps.tile([128, 512], F32, tag="psc", bufs=3,
                              padded_shape=[128, 1024],
                              name=f"pwo_{t2c}_{oc}")
                for ic in range(8):
                    nc.tensor.matmul(
                        out=pmm[:], lhsT=wo_c[:, ic, :],
                        rhs=ctx_t[ic][:],
                        start=(ic == 0), stop=(ic == 7),
                    )
                x_ = gen.tile([128, 512], F32, tag="xres", bufs=2,
                              name=f"x_{t2c}_{oc}")
                b0 = t2c * 2
                nc.sync.dma_start(
                    out=x_[:],
                    in_=xts[oc * 128:(oc + 1) * 128,
                            b0:b0 + 2, :].rearrange("p b s -> p (b s)"),
                )
                r1 = gen.tile([128, 512], F32R, tag="r1t", bufs=9,
                              name=f"r1_{t2c}_{oc}")
                nc.vector.scalar_tensor_tensor(
                    out=r1[:], in0=pmm[:], scalar=wob_sb[:, oc:oc + 1],
                    in1=x_[:], op0=ALU.add, op1=ALU.add,
                )
                r1_t.append(r1)

            def load_ctx(t2c):
                b0 = t2c * 2
                ctx_t = []
                for ic in range(8):
                    c_ = gen.tile([128, 512], BF16, tag="ctx", bufs=8,
                                  name=f"c_{t2c}_{ic}")
                    for half in range(2):
                        nc.sync.dma_start(
                            out=c_[:, half * 256:(half + 1) * 256],
                            in_=a2a_out[b0 + half][ic, :, :, :].rearrange(
                                "hl v s -> (hl v) s"
                            ),
                        )
                    ctx_t.append(c_)
                return ctx_t

            def emit_mlp1(t2c, x1_t):
                h_t = []
                for fc in range(32):
                    if t2c == 0 and fc in pre_w1:
                        w1_c = pre_w1.pop(fc)
                    else:
                        w1_c = gen.tile([128, 8, 128], BF16, tag="w1_c",
                                        bufs=3, name=f"w1_c_{t2c}_{fc}")
                        nc.sync.dma_start(
                            out=w1_c[:],
                            in_=w1tr[fc, :, :, :].rearrange(
                                "ic p j -> p ic j"),
                        )
                    pmm = ps.tile([128, 512], F32, tag="psc", bufs=3,
                                  padded_shape=[128, 1024],
                                  name=f"ph_{t2c}_{fc}")
                    for ic in range(8):
                        nc.tensor.matmul(
                            out=pmm[:], lhsT=w1_c[:, ic, :], rhs=x1_t[ic][:],
                            start=(ic == 0), stop=(ic == 7),
                        )
                    h_ = gen.tile([128, 512], BF16, tag="ht", bufs=32,
                                  name=f"h_{t2c}_{fc}")
                    nc.vector.tensor_scalar(
                        out=h_[:], in0=pmm[:],
                        scalar1=b1_sb[:, fc:fc + 1], scalar2=0.0,
                        op0=ALU.add, op1=ALU.max,
                    )
                    h_t.append(h_)
                return h_t

            def emit_mlp2(t2c, h_t, x1_t):
                r2_t = []
                for oc in range(8):
                    pmm = ps.tile([128, 512], F32, tag="psc", bufs=3,
                                  padded_shape=[128, 1024],
                                  name=f"pm_{t2c}_{oc}")
                    for half in range(2):
                        if t2c == 0 and oc == 0 and half in pre_w2:
                            w2_c = pre_w2.pop(half)
                        else:
                            w2_c = gen.tile(
                                [128, 16, 128], BF16, tag="w2_c", bufs=2,
                                name=f"w2_c_{t2c}_{oc}_{half}")
                            nc.sync.dma_start(
                                out=w2_c[:],
                                in_=w2tr[oc, half * 16:(half + 1) * 16, :, :]
                                .rearrange("fc p j -> p fc j"),
                            )
                        for f2 in range(16):
                            fc = half * 16 + f2
                            nc.tensor.matmul(
                                out=pmm[:], lhsT=w2_c[:, f2, :],
                                rhs=h_t[fc][:],
                                start=(fc == 0), stop=(fc == 31),
                            )
                    r2p = gen.tile([128, 512], F32, tag="lnw", bufs=3,
                                   name=f"r2p_{t2c}_{oc}")
                    nc.vector.scalar_tensor_tensor(
                        out=r2p[:], in0=x1_t[oc][:],
                        scalar=ln_sb["ln1g"][:, oc:oc + 1],
                        in1=pmm[:], op0=ALU.mult, op1=ALU.add,
                    )
                    r2 = gen.tile([128, 512], F32R, tag="u2k", bufs=16,
                                  name=f"r2_{t2c}_{oc}")
                    nc.vector.tensor_scalar_add(
                        out=r2[:], in0=r2p[:], scalar1=b2_sb[:, oc:oc + 1],
                    )
                    r2_t.append(r2)
                return r2_t

            def emit_out(t2c, o_t):
                b0 = t2c * 2
                for oc in range(8):
                    nc.sync.dma_start(
                        out=ot[oc * 128:(oc + 1) * 128,
                               b0:b0 + 2, :].rearrange("p b s -> p (b s)"),
                        in_=o_t[oc][:],
                    )

            # =====================================================
            # Main schedule
            # =====================================================
            # Prologue: full QKV + V-transpose for batch 0
            for sc in range(4):
                xt_t = load_xt(0, sc)
                for pi, name in enumerate(("wq", "wk", "wv")):
                    emit_p1_proj(0, sc, pi, name, xt_t)
                for lc in range(4 * sc, 4 * sc + 4):
                    emit_p2_chunk(0, lc)

            ctx0_t = None
            r1_0 = []
            pending = None
            for b in range(B):
                for sc in range(4):
                    extra = {}
                    if b < 3:
                        # QKV of batch b+1, spread across this group
                        xt_box = []

                        def load_thunk(b=b, sc=sc, box=xt_box):
                            box.append(load_xt(b + 1, sc))

                        extra[0] = [load_thunk]
                        extra[1] = [lambda b=b, sc=sc, box=xt_box:
                                    emit_p1_proj(b + 1, sc, 0, "wq", box[0])]
                        extra[3] = [lambda b=b, sc=sc, box=xt_box:
                                    emit_p1_proj(b + 1, sc, 1, "wk", box[0])]
                        extra[5] = [lambda b=b, sc=sc, box=xt_box:
                                    emit_p1_proj(b + 1, sc, 2, "wv", box[0])]
                    else:
                        # WO of back-half chunk0 (a2a 0/1 done long ago)
                        extra[1] = [lambda sc=sc:
                                    emit_wo_oc(0, 2 * sc, ctx0_t, r1_0)]
                        extra[5] = [lambda sc=sc:
                                    emit_wo_oc(0, 2 * sc + 1, ctx0_t, r1_0)]
                    pctx, recips = emit_attn_sc(b, sc, extra, pending)
                    pending = (b, sc, pctx, recips)
                # flush last group's softmax before the a2a
                emit_softmax_norm(*pending)
                pending = None
                nc.gpsimd.collective_compute(
                    "AllToAll",
                    ALU.bypass,
                    replica_groups=[list(range(NC))],
                    ins=[a2a_in[b][:].opt()],
                    outs=[a2a_out[b][:].opt()],
                )
                if b < 3:
                    # V-transposes for batch b+1 in the inter-batch gap
                    for lc in range(16):
                        emit_p2_chunk(b + 1, lc)
                if b == 1:
                    # ctx loads for back-half chunk0 (consumed in batch 3)
                    ctx0_t = load_ctx(0)

            # Back half: chunk0 LN1+MLP covers a2a(b3) latency, then chunk1
            st1_0 = ln_stats(r1_0, "l1c0")
            x1_0 = ln_norm(st1_0, r1_0, None, None, BF16, True, "l1c0")
            h_0 = emit_mlp1(0, x1_0)
            ctx1_t = load_ctx(1)
            r1_1 = []
            for oc in range(8):
                emit_wo_oc(1, oc, ctx1_t, r1_1)
            st1_1 = ln_stats(r1_1, "l1c1")
            x1_1 = ln_norm(st1_1, r1_1, None, None, BF16, True, "l1c1")
            r2_0 = emit_mlp2(0, h_0, x1_0)
            st2_0 = ln_stats(r2_0, "l2c0")
            h_1 = emit_mlp1(1, x1_1)
            def out_cb0(ic, o_):
                nc.sync.dma_start(
                    out=ot[ic * 128:(ic + 1) * 128,
                           0:2, :].rearrange("p b s -> p (b s)"),
                    in_=o_[:],
                )
            ln_norm(st2_0, r2_0, ln_sb["ln2g"], ln_sb["ln2b"], F32,
                    False, "l2c0", out_cb=out_cb0)
            r2_1 = emit_mlp2(1, h_1, x1_1)
            st2_1 = ln_stats(r2_1, "l2c1")
            def out_cb1(ic, o_):
                nc.sync.dma_start(
                    out=ot[ic * 128:(ic + 1) * 128,
                           2:4, :].rearrange("p b s -> p (b s)"),
                    in_=o_[:],
                )
            ln_norm(st2_1, r2_1, ln_sb["ln2g"], ln_sb["ln2b"], F32,
                    False, "l2c1", out_cb=out_cb1)

    nc.compile()
    return nc


# ------------------------------------------------------------------
# Host side
# ------------------------------------------------------------------
def _get_runner():
    if "runner" in _CACHE:
        return _CACHE["runner"]
    import jax
    from jax.sharding import Mesh, PartitionSpec
    try:
        from jax.experimental.shard_map import shard_map
    except ImportError:
        from jax.shard_map import shard_map
    from concourse import bass2jax
    from concourse.bass2jax import _bass_exec_p, install_neuronx_cc_hook

    nc = _build_nc()
    install_neuronx_cc_hook()
    partition_name = nc.partition_id_tensor.name if nc.partition_id_tensor else None
    in_names, out_names, out_avals, zero_outs = [], [], [], []
    for alloc in nc.m.functions[0].allocations:
        if not isinstance(alloc, mybir.MemoryLocationSet):
            continue
        name = alloc.memorylocations[0].name
        if alloc.kind == "ExternalInput":
            if name != partition_name:
                in_names.append(name)
        elif alloc.kind == "ExternalOutput":
            out_names.append(name)
            shape = tuple(alloc.tensor_shape)
            dtype = mybir.dt.np(alloc.dtype)
            out_avals.append(jax.core.ShapedArray(shape, dtype))
            zero_outs.append(np.zeros(shape, dtype))
    n_params = len(in_names)
    all_in_names = list(in_names) + list(out_names)
    if partition_name is not None:
        all_in_names.append(partition_name)

    def _body(*args):
        operands = list(args)
        if partition_name is not None:
            operands.append(bass2jax.partition_id_tensor())
        outs = _bass_exec_p.bind(
            *operands,
            out_avals=tuple(out_avals),
            in_names=tuple(all_in_names),
            out_names=tuple(out_names),
            lowering_input_output_aliases=(),
            sim_require_finite=True,
            sim_require_nnan=True,
            nc=nc,
        )
        return tuple(outs)

    donate = tuple(range(n_params, n_params + len(out_names)))
    devices = jax.devices()[:NC]
    mesh = Mesh(np.asarray(devices), ("core",))
    in_specs = (PartitionSpec("core"),) * (n_params + len(out_names))
    out_specs = (PartitionSpec("core"),) * len(out_names)
    fn = jax.jit(
        shard_map(_body, mesh=mesh, in_specs=in_specs, out_specs=out_specs,
                  check_rep=False),
        donate_argnums=donate, keep_unused=True,
    )

    class R:
        pass

    r = R()
    r.fn = fn
    r.in_names = in_names
    r.out_names = out_names
    r.out_avals = out_avals
    _CACHE["runner"] = r
    return r


def _prep_in_maps(X, WQ_w, WQ_b, WK_w, WK_b, WV_w, WV_b, WO_w, WO_b,
                  ln1_g, ln1_b, W1, b1, W2, b2, ln2_g, ln2_b):
    import ml_dtypes
    f = np.float32
    bf = ml_dtypes.bfloat16
    XT = np.ascontiguousarray(X.transpose(2, 1, 0)).astype(f)  # [DM,B,S]
    wotr = np.ascontiguousarray(
        WO_w.reshape(8, 128, 8, 128).transpose(0, 2, 3, 1)).astype(bf)
    W1f = (W1 * ln1_g[None, :]).astype(np.float64)
    b1f = (b1 + W1 @ ln1_b).astype(f)
    w1tr = np.ascontiguousarray(
        W1f.reshape(32, 128, 8, 128).transpose(0, 2, 3, 1)).astype(bf)
    w2tr = np.ascontiguousarray(
        W2.reshape(8, 128, 32, 128).transpose(0, 2, 3, 1)).astype(bf)
    wob_t = np.ascontiguousarray(WO_b.reshape(8, 128).T).astype(f)
    b1_t = np.ascontiguousarray(b1f.reshape(32, 128).T).astype(f)
    b2f = (b2 + ln1_b).astype(f)
    b2_t = np.ascontiguousarray(b2f.reshape(8, 128).T).astype(f)
    ln1g_t = np.ascontiguousarray(ln1_g.reshape(8, 128).T).astype(f)
    ln1b_t = np.ascontiguousarray(ln1_b.reshape(8, 128).T).astype(f)
    ln2g_t = np.ascontiguousarray(ln2_g.reshape(8, 128).T).astype(f)
    ln2b_t = np.ascontiguousarray(ln2_b.reshape(8, 128).T).astype(f)

    in_maps = []
    for c in range(NC):
        h0 = 2 * c
        # [2,DK,DM] -> [DM, 128]: W2h[j, hl*64+k] = W[h0+hl, k, j]
        wq2 = WQ_w[h0:h0 + 2].reshape(128, DM).T / 8.0
        wk2 = WK_w[h0:h0 + 2].reshape(128, DM).T
        wv2 = WV_w[h0:h0 + 2].reshape(128, DM).T
        # [8,128,128] layout: [ic, p, j] = W2h[ic*128+p, j]
        wqt = np.ascontiguousarray(wq2.reshape(8, 128, 128)).astype(bf)
        wkt = np.ascontiguousarray(wk2.reshape(8, 128, 128)).astype(bf)
        wvt = np.ascontiguousarray(wv2.reshape(8, 128, 128)).astype(bf)
        bq = WQ_b[h0:h0 + 2].reshape(128) / 8.0
        bk = WK_b[h0:h0 + 2].reshape(128)
        bv = WV_b[h0:h0 + 2].reshape(128)
        bqkv = np.stack([bq, bk, bv], axis=1).astype(f)
        in_maps.append({
            "xt": XT.astype(bf),
            "xts": np.ascontiguousarray(XT[:, :, c * SS:(c + 1) * SS]),
            "wqt": wqt, "wkt": wkt, "wvt": wvt, "bqkv": bqkv,
            "wotr": wotr, "wob": wob_t,
            "w1tr": w1tr, "b1": b1_t, "w2tr": w2tr, "b2": b2_t,
            "ln1g": ln1g_t, "ln1b": ln1b_t, "ln2g": ln2g_t, "ln2b": ln2b_t,
        })
    return in_maps


def run_in_maps(in_maps):
    """Run the compiled kernel on prepared in_maps; returns list of out dicts."""
    import jax
    r = _get_runner()
    n = NC
    per_core = [[np.asarray(m[name]) for name in r.in_names] for m in in_maps]
    concat_in = [
        np.concatenate([per_core[c][i] for c in range(n)], axis=0)
        for i in range(len(r.in_names))
    ]
    concat_zeros = [
        np.zeros((n * a.shape[0], *a.shape[1:]), a.dtype) for a in r.out_avals
    ]
    out_arrs = r.fn(*concat_in, *concat_zeros)
    out_arrs = [np.asarray(a) for a in out_arrs]
    return [
        {name: out_arrs[i].reshape(n, *r.out_avals[i].shape)[c]
         for i, name in enumerate(r.out_names)}
        for c in range(n)
    ]


def kernel(**inputs):
    in_maps = _prep_in_maps(**inputs)
    results = run_in_maps(in_maps)
    # assemble: each core's ot is [DM, B, SS] covering s in [c*SS,(c+1)*SS)
    ot_full = np.concatenate([results[c]["ot"] for c in range(NC)], axis=2)
    # [DM, B, S] -> [S, B, DM]
    return np.ascontiguousarray(ot_full.transpose(2, 1, 0))
